# revision 37
# baseline (speedup 1.0000x reference)
"""BiMamba block kernel for 8 Trainium2 NeuronCores.

Sharding: core = 2*sample + direction (4 samples x 2 scan directions).
Each core runs the full mamba for its (sample, direction).

The selective scan dominates on DVE (16 states x 4096 cols at ~2.1
ns/col is irreducible), so the kernel keeps the DVE queue dense and
hides everything else under it:

- All activations use only the natural_log_exp ACT table set: silu is
  computed as v*sigmoid(v) with sigmoid(v) = exp(-softplus(-v)), so no
  table reload ever interrupts the exp stream of the scan.
- Front: fused in-proj + causal conv matmuls with one-chunk lookahead
  on the PE queue (chunk c+1's input matmuls are emitted before chunk
  c's xc-dependent ones, so the PE never idles on a silu).
- Scan: 3 segments (2048/1536/512).  The DVE state loop is software
  pipelined (scan_s | dbx_{s+1} | g_s) so no two adjacent DVE ops are
  dependent.  B/C rows arrive via DRAM partition-broadcast DMAs issued
  6 states ahead.
- Exchange: after each segment the out-projection rows are AllGathered
  with the pair core (rank order = [dir0; dir1], matching the
  reference's un-unflipped y2 concat); the 3x3 conv runs locally.
  A dummy 8-core AllReduce early in the kernel absorbs core launch
  skew (~45-60 us) so tail collectives don't pay it.
- Conv is cut into row-pieces such that only rows 55-63 depend on the
  last exchange; BN stats for pieces 0-6 AllReduce early.  invstd via
  ln/exp (no sqrt table load).
"""
import os
import sys

for _p in ("/opt/trn_rl_repo", "/root/.axon_site/_ro/trn_rl_repo"):
    if os.path.isdir(_p):
        if _p not in sys.path:
            sys.path.insert(0, _p)
        break

import ml_dtypes
import numpy as np

# The agent image's antenv lacks axon_hooks; inject it so trace=True can
# capture NTFF profiles (used by test.py for HW timing, not for grading).
try:
    import antenv.axon_hooks  # noqa: F401
except ImportError:
    try:
        import types as _types

        from trn_agent_boot.trn_boot import _ntff_profile_via_ctypes

        _hook = _ntff_profile_via_ctypes("/opt/axon/libaxon_pjrt.so")
        _m = _types.ModuleType("antenv.axon_hooks")
        _m.get_axon_ntff_profile_hook = lambda: _hook
        _m.set_axon_ntff_profile_hook = lambda h: None
        sys.modules["antenv.axon_hooks"] = _m
    except Exception:
        pass

import concourse.bass as bass
import concourse.mybir as mybir
from concourse import bacc
from concourse import bass_utils
from concourse.masks import make_identity
from concourse.tile import TileContext

F32 = mybir.dt.float32
BF16 = mybir.dt.bfloat16
AF = mybir.ActivationFunctionType
OP = mybir.AluOpType

B, C, H, W = 4, 64, 64, 64
L = H * W          # 4096
DI = 128           # d_inner
DS = 16            # d_state
DTR = 4            # dt_rank
DCONV = 4
NCORE = 8
CH = 512           # matmul free-dim chunk
NCH = L // CH      # 8
RPC = CH // W      # output rows per chunk (8)

SEGS = ((0, 2048), (2048, 3584), (3584, 4096))
SEG_CHUNKS = ((0, 1, 2, 3), (4, 5, 6), (7,))
NSEG = len(SEGS)
WAVES = ((0, (0, 1, 2, 3)), (1, (4, 5, 6)), (2, (7,)))
# conv pieces (row ranges): pieces 0-6 need only waves 0-1; pieces 7-8
# (rows 55-63, PSUM-bank-sized) are the only ones gated on wave 2
CONV_PIECES = ((0, 8), (8, 16), (16, 24), (24, 32), (32, 40), (40, 48),
               (48, 55), (55, 63), (63, 64))
NPIECE = len(CONV_PIECES)

# blob_h layout (bf16): c3w | owT | bigT | bcwT | wk0..3 | zwT | rwT
OFF_C3W = 0
OFF_OWT = OFF_C3W + 9 * C
OFF_BIG = OFF_OWT + C
OFF_BCW = OFF_BIG + 128
OFF_WK = OFF_BCW + 32
OFF_ZWT = OFF_WK + 4 * 128
OFF_RWT = OFF_ZWT + 128
BH_COLS = OFF_RWT + C
BF_COLS = 32


def _build():
    nc = bacc.Bacc(target_bir_lowering=False, debug=False, num_devices=NCORE)

    def din(name, shape, dtype=F32):
        return nc.dram_tensor(name, shape, dtype, kind="ExternalInput")

    F32R = mybir.dt.float32r
    x_loc = din("x_loc", [C, L], BF16)
    blob_f = din("blob_f", [128, BF_COLS], F32)
    blob_h = din("blob_h", [128, BH_COLS], BF16)

    out_d = nc.dram_tensor("out", [C, L], F32, kind="ExternalOutput")

    with TileContext(nc) as tc:
        with tc.tile_pool(name="pers", bufs=1) as pers:
            # ---- params arrive as two packed blobs ----
            p_bf = pers.tile([128, BF_COLS], F32)
            p_bh = pers.tile([128, BH_COLS], BF16)
            nc.sync.dma_start(p_bf[:], blob_f[:])
            nc.sync.dma_start(p_bh[:], blob_h[:])
            p_c1b = p_bf[:, 0:1]
            p_dtb = p_bf[:, 1:2]
            p_A = p_bf[:, 2:18]
            p_D = p_bf[:, 18:19]
            p_c3b = p_bf[:C, 19:20]
            p_rb = p_bf[:C, 20:21]
            p_bng = p_bf[:C, 21:22]
            p_bnb = p_bf[:C, 22:23]
            p_nc1b = p_bf[:, 23:24]
            p_c3w = p_bh[:, OFF_C3W:OFF_OWT]
            p_owT = p_bh[:, OFF_OWT:OFF_BIG]
            p_bigT = p_bh[:, OFF_BIG:OFF_BCW]
            p_bcwT = p_bh[:, OFF_BCW:OFF_WK]
            p_wk = [p_bh[:, OFF_WK + 128 * k:OFF_WK + 128 * (k + 1)]
                    for k in range(DCONV)]
            p_zwT = p_bh[:, OFF_ZWT:OFF_ZWT + 128]
            p_rwT = p_bh[:, OFF_RWT:OFF_RWT + C]

            ident = pers.tile([128, 128], F32)
            make_identity(nc, ident[:])
            ident_g = pers.tile([128, 128], BF16)
            nc.vector.tensor_copy(ident_g[:], ident[:])

            # DRAM staging for B/C rows (DMA partition-broadcast needs a
            # DRAM source)
            bc_dram = nc.dram_tensor("bc_stage", [2 * DS, L], BF16)

            x_pad = pers.tile([64, 3 + L], BF16)
            nc.gpsimd.memset(x_pad[:, 0:3], 0.0)
            # split load so the front chunks start without waiting for
            # the full x (chunk 3's last tap reads through x col 2048)
            nc.sync.dma_start(x_pad[:, 3:3 + 2064], x_loc[:, 0:2064])
            nc.sync.dma_start(x_pad[:, 3 + 2064:3 + L], x_loc[:, 2064:L])

            with tc.tile_pool(name="smid", bufs=1) as smid, \
                 tc.tile_pool(name="ps", bufs=4, space="PSUM") as psp, \
                 tc.tile_pool(name="psy", bufs=4, space="PSUM") as psy, \
                 tc.tile_pool(name="sl_e", bufs=2) as plex, \
                 tc.tile_pool(name="sl_a", bufs=3) as pla, \
                 tc.tile_pool(name="sl_b", bufs=7) as plb, \
                 tc.tile_pool(name="sl_x", bufs=3) as plx, \
                 tc.tile_pool(name="sl_h", bufs=3) as plh, \
                 tc.tile_pool(name="sl_c", bufs=7) as plc, \
                 tc.tile_pool(name="sl_g", bufs=3) as plg, \
                 tc.tile_pool(name="sl_f", bufs=2) as plf, \
                 tc.tile_pool(name="dram", bufs=1, space="DRAM") as dr:
                z_sil = smid.tile([DI, L], BF16)
                dtv = smid.tile([DI, L], BF16)
                dtxc = smid.tile([DI, L], BF16)
                xcd = smid.tile([DI, L], BF16)
                xc = smid.tile([DI, L], BF16)
                carry = smid.tile([DI, DS], F32)

                ympad = smid.tile([128, H + 2, W + 2], BF16)
                nc.gpsimd.memset(ympad[:], 0.0)
                # res rows 0-63, conv rows 64-127 share one tile
                rescv = smid.tile([128, L], BF16)
                res_sb = rescv[0:C]
                conv_sb = rescv[C:128]
                stats_m = smid.tile([C, NPIECE], F32)
                stats_v = smid.tile([C, NPIECE], F32)
                PAIRS = [[0, 1], [2, 3], [4, 5], [6, 7]]
                G8 = [[0, 1, 2, 3, 4, 5, 6, 7]]

                cc_ins, cc_outs = [], []
                for wi, (_, cvs) in enumerate(WAVES):
                    cc_ins.append(dr.tile([C, len(cvs) * CH], BF16,
                                          name=f"cci{wi}"))
                    cc_outs.append(dr.tile([128, len(cvs) * CH], BF16,
                                           name=f"cco{wi}"))
                st_in_a = dr.tile([C, 2], F32, name="st_in_a")
                st_in_b = dr.tile([C, 2], F32, name="st_in_b")
                sync_in = dr.tile([C, 2], F32, name="sync_in")
                st_out_a = nc.dram_tensor("st_out_a", [C, 2], F32,
                                          addr_space="Shared")
                st_out_b = nc.dram_tensor("st_out_b", [C, 2], F32,
                                          addr_space="Shared")
                sync_out = nc.dram_tensor("sync_out", [C, 2], F32,
                                          addr_space="Shared")

                def sigmoid_mul(dst_sl, ps_t, bias, nbias):
                    """dst = (v+b)*sigmoid(v+b) with sigmoid computed as
                    exp(-ln(1+exp(-(v+b)))) -- exp/ln only, one table set."""
                    t = plf.tile([DI, CH], BF16, tag="sg")
                    if nbias is not None:
                        nc.scalar.activation(t[:], ps_t[:DI], AF.Exp,
                                             scale=-1.0, bias=nbias)
                    else:
                        nc.scalar.activation(t[:], ps_t[:DI], AF.Exp,
                                             scale=-1.0)
                    nc.scalar.activation(t[:], t[:], AF.Ln, bias=1.0)
                    nc.scalar.activation(t[:], t[:], AF.Exp, scale=-1.0)
                    nc.vector.scalar_tensor_tensor(
                        dst_sl, ps_t[:DI],
                        bias if bias is not None else 0.0,
                        t[:], op0=OP.add, op1=OP.mult)

                def front_in(c):
                    """Input-dependent matmuls for chunk c (no xc dep)."""
                    ps = psp.tile([128, CH], F32, tag="ps", name=f"fi{c}")
                    for k in range(DCONV):
                        nc.tensor.matmul(ps[:DI], p_wk[k][:C],
                                         x_pad[:, c * CH + k:c * CH + k + CH],
                                         start=(k == 0), stop=(k == DCONV - 1))
                    ps2 = psp.tile([128, CH], F32, tag="ps", name=f"fz{c}")
                    nc.tensor.matmul(ps2[:DI], p_zwT[:C],
                                     x_pad[:, 3 + c * CH:3 + (c + 1) * CH],
                                     start=True, stop=True)
                    return ps, ps2

                def front_mid(c, ps, ps2):
                    """silus for chunk c (ACT+DVE)."""
                    sl = slice(c * CH, (c + 1) * CH)
                    sigmoid_mul(xc[:, sl], ps, p_c1b, p_nc1b)
                    sigmoid_mul(z_sil[:, sl], ps2, None, None)

                def front_out(c, with_ln):
                    """xc-dependent projections for chunk c."""
                    sl = slice(c * CH, (c + 1) * CH)
                    ps3 = psp.tile([128, CH], F32, tag="ps", name=f"fd{c}")
                    nc.tensor.matmul(ps3[:DI], p_bigT[:], xc[:, sl],
                                     start=True, stop=True)
                    nc.scalar.activation(dtv[:, sl], ps3[:DI], AF.Exp,
                                         bias=p_dtb)
                    ps4 = psp.tile([128, CH], F32, tag="ps", name=f"fb{c}")
                    nc.tensor.matmul(ps4[:2 * DS], p_bcwT[:], xc[:, sl],
                                     start=True, stop=True)
                    bch = plb.tile([2 * DS, CH], BF16, tag="bch")
                    nc.scalar.copy(bch[:], ps4[:2 * DS])
                    nc.sync.dma_start(bc_dram[:, sl], bch[:])
                    if with_ln:
                        nc.scalar.activation(dtv[:, sl], dtv[:, sl], AF.Ln,
                                             bias=1.0)
                        nc.vector.tensor_mul(dtxc[:, sl], dtv[:, sl],
                                             xc[:, sl])
                        nc.scalar.activation(xcd[:, sl], xc[:, sl],
                                             AF.Copy, scale=p_D)

                def front_chunk(c, with_ln):
                    ps, ps2 = front_in(c)
                    front_mid(c, ps, ps2)
                    front_out(c, with_ln)

                def finish_front(cs):
                    hsl = slice(cs[0] * CH, (cs[-1] + 1) * CH)
                    nc.scalar.activation(dtv[:, hsl], dtv[:, hsl], AF.Ln,
                                         bias=1.0)
                    for c in cs:
                        sl = slice(c * CH, (c + 1) * CH)
                        nc.vector.tensor_mul(dtxc[:, sl], dtv[:, sl],
                                             xc[:, sl])
                        nc.scalar.activation(xcd[:, sl], xc[:, sl],
                                             AF.Copy, scale=p_D)

                def wave(wi):
                    """Out-projection + pair AllGather + ympad write +
                    residual for the wave's chunks.  AllGather output is
                    rank-ordered, so both cores get [dir0; dir1]."""
                    cvs = WAVES[wi][1]
                    stage = plex.tile([C, len(cvs) * CH], BF16,
                                      tag="stage", name=f"stage{wi}")
                    for j, cix in enumerate(cvs):
                        sl = slice(cix * CH, (cix + 1) * CH)
                        ssl = slice(j * CH, (j + 1) * CH)
                        yg = plf.tile([DI, CH], BF16, tag="yg")
                        nc.vector.tensor_mul(yg[:], y_ps[cix][:DI],
                                             z_sil[:, sl])
                        po = psp.tile([128, CH], F32, tag="ps",
                                      name=f"po{cix}")
                        nc.tensor.matmul(po[:C], p_owT[:], yg[:],
                                         start=True, stop=True)
                        nc.scalar.copy(stage[:, ssl], po[:C])
                        psr = psp.tile([128, CH], F32, tag="ps",
                                       name=f"rs{cix}")
                        nc.tensor.matmul(psr[:C], p_rwT[:C],
                                         x_pad[:, 3 + cix * CH:
                                               3 + (cix + 1) * CH],
                                         start=True, stop=True)
                        nc.scalar.activation(res_sb[:, sl], psr[:C],
                                             AF.Identity, bias=p_rb)
                    nc.sync.dma_start(cc_ins[wi][:], stage[:])
                    nc.gpsimd.collective_compute(
                        "AllGather", OP.bypass, replica_groups=PAIRS,
                        ins=[cc_ins[wi][:].opt()], outs=[cc_outs[wi][:].opt()])
                    r0 = cvs[0] * RPC
                    nrows = len(cvs) * RPC
                    nc.sync.dma_start(
                        ympad[:, 1 + r0:1 + r0 + nrows, 1:1 + W],
                        cc_outs[wi][:].rearrange("p (r w) -> p r w", w=W))

                def conv3_piece(i):
                    r0, r1 = CONV_PIECES[i]
                    nr = r1 - r0
                    ps = psp.tile([128, nr * W], F32, tag="ps",
                                  name=f"cv{i}")
                    ps3 = ps[:C].rearrange("p (r w) -> p r w", w=W)
                    n = 0
                    for ky in range(3):
                        for kx in range(3):
                            nc.tensor.matmul(
                                ps3[:],
                                p_c3w[:, (ky * 3 + kx) * C:
                                      (ky * 3 + kx + 1) * C],
                                ympad[:, r0 + ky:r0 + ky + nr, kx:kx + W],
                                start=(n == 0), stop=(n == 8))
                            n += 1
                    sl = slice(r0 * W, r1 * W)
                    flat = ps3.rearrange("p r w -> p (r w)")
                    nc.scalar.activation(conv_sb[:, sl], flat,
                                         AF.Identity, bias=p_c3b,
                                         accum_out=stats_m[:, i:i + 1])
                    sq = plf.tile([C, nr * W], BF16, tag="sq")
                    nc.scalar.activation(sq[:], conv_sb[:, sl],
                                         AF.Square,
                                         accum_out=stats_v[:, i:i + 1])

                stats_a = smid.tile([C, 2], F32)

                def stats_early():
                    nc.vector.tensor_reduce(stats_a[:, 0:1],
                                            stats_m[:, 0:7],
                                            axis=mybir.AxisListType.X,
                                            op=OP.add)
                    nc.vector.tensor_reduce(stats_a[:, 1:2],
                                            stats_v[:, 0:7],
                                            axis=mybir.AxisListType.X,
                                            op=OP.add)
                    nc.sync.dma_start(st_in_a[:], stats_a[:])
                    nc.gpsimd.collective_compute(
                        "AllReduce", OP.add, replica_groups=G8,
                        ins=[st_in_a[:].opt()], outs=[st_out_a[:].opt()])

                y_ps = {}
                bc_q = {}
                PREF = 6

                def bc_issue(q, s):
                    t0, t1 = SEGS[q]
                    SEG = t1 - t0
                    qsl = slice(t0, t1)
                    bbc = plb.tile([DI, SEG], BF16, tag="bbc",
                                   name=f"bbc{q}_{s}")
                    nc.sync.dma_start(
                        bbc[:],
                        bc_dram[s:s + 1, qsl].to_broadcast((DI, SEG)))
                    cbc = plc.tile([DI, SEG], BF16, tag="cbc",
                                   name=f"cbc{q}_{s}")
                    nc.sync.dma_start(
                        cbc[:],
                        bc_dram[DS + s:DS + s + 1, qsl].to_broadcast(
                            (DI, SEG)))
                    bc_q[(q, s)] = (bbc, cbc)

                def seg_prefetch(q):
                    for s in range(PREF):
                        bc_issue(q, s)

                def seg_scan(q, work):
                    t0, t1 = SEGS[q]
                    SEG = t1 - t0
                    qsl = slice(t0, t1)
                    for cix in SEG_CHUNKS[q]:
                        yp = psy.tile([128, CH], F32, tag="yps",
                                      name=f"y{cix}")
                        nc.tensor.matmul(yp[:DI], ident_g[:],
                                         xcd[:, cix * CH:(cix + 1) * CH],
                                         start=True, stop=False)
                        y_ps[cix] = yp
                    # software-pipelined state loop: no two adjacent DVE
                    # ops are data-dependent (scan_s | dbx_{s+1} | g_s)
                    da0 = pla.tile([DI, SEG], BF16, tag="da")
                    nc.scalar.activation(da0[:], dtv[:, qsl], AF.Exp,
                                         scale=p_A[:, 0:1])
                    bbc0, _ = bc_q[(q, 0)]
                    dbx_n = plx.tile([DI, SEG], BF16, tag="dbx")
                    nc.vector.tensor_mul(dbx_n[:], dtxc[:, qsl], bbc0[:])
                    da_n = da0
                    for s in range(DS):
                        da, dbx = da_n, dbx_n
                        _, cbc = bc_q.pop((q, s))
                        if s + 1 < DS:
                            da_n = pla.tile([DI, SEG], BF16, tag="da")
                            nc.scalar.activation(da_n[:], dtv[:, qsl],
                                                 AF.Exp,
                                                 scale=p_A[:, s + 1:s + 2])
                        h = plh.tile([DI, SEG], BF16, tag="h")
                        init = 0.0 if q == 0 else carry[:, s:s + 1]
                        nc.vector.tensor_tensor_scan(h[:], da[:], dbx[:],
                                                     init, op0=OP.mult,
                                                     op1=OP.add)
                        if s + 1 < DS:
                            bbc_n, _ = bc_q[(q, s + 1)]
                            dbx_n = plx.tile([DI, SEG], BF16, tag="dbx")
                            nc.vector.tensor_mul(dbx_n[:], dtxc[:, qsl],
                                                 bbc_n[:])
                        g = plg.tile([DI, SEG], BF16, tag="g")
                        nc.vector.tensor_mul(g[:], h[:], cbc[:])
                        if q < NSEG - 1:
                            nc.vector.tensor_copy(carry[:, s:s + 1],
                                                  h[:, SEG - 1:SEG])
                        for j, cix in enumerate(SEG_CHUNKS[q]):
                            nc.tensor.matmul(
                                y_ps[cix][:DI], ident_g[:],
                                g[:, j * CH:(j + 1) * CH],
                                start=False, stop=(s == DS - 1))
                        if s + PREF < DS:
                            bc_issue(q, s + PREF)
                        if s in work:
                            work[s]()

                # =========== emission ===========
                # front with one-chunk PE lookahead: chunk c+1's input
                # matmuls are emitted before chunk c's xc-dependent ones
                pends = {0: front_in(0)}
                # dummy 8-core sync: absorbs core launch skew while the
                # front runs, so tail collectives don't pay it
                nc.gpsimd.memset(stats_a[:], 0.0)
                nc.sync.dma_start(sync_in[:], stats_a[:])
                nc.gpsimd.collective_compute(
                    "AllReduce", OP.add, replica_groups=G8,
                    ins=[sync_in[:].opt()], outs=[sync_out[:].opt()])
                for c in range(4):
                    if c + 1 < 4:
                        pends[c + 1] = front_in(c + 1)
                    front_mid(c, *pends.pop(c))
                    front_out(c, False)
                finish_front((0, 1, 2, 3))

                seg_prefetch(0)
                seg_scan(0, {2: lambda: front_chunk(4, True),
                             5: lambda: front_chunk(5, True),
                             8: lambda: front_chunk(6, True),
                             11: lambda: front_chunk(7, True),
                             13: lambda: seg_prefetch(1)})
                wave(0)
                seg_scan(1, {6: lambda: conv3_piece(0),
                             9: lambda: conv3_piece(1),
                             12: lambda: conv3_piece(2),
                             13: lambda: seg_prefetch(2)})
                wave(1)
                seg_scan(2, {})
                wave(2)
                # conv pieces 3-6 hide the last exchange's latency on PE
                conv3_piece(3)
                conv3_piece(4)
                conv3_piece(5)
                conv3_piece(6)
                stats_early()
                conv3_piece(7)
                conv3_piece(8)

                # ---- batch stats AllReduces + BN + residual + leaky ----
                tl = smid
                stot = tl.tile([C, 2], F32)
                stot_b = tl.tile([C, 2], F32)
                stats = tl.tile([C, 2], F32)
                nc.vector.tensor_reduce(stats[:, 0:1], stats_m[:, 7:9],
                                        axis=mybir.AxisListType.X, op=OP.add)
                nc.vector.tensor_reduce(stats[:, 1:2], stats_v[:, 7:9],
                                        axis=mybir.AxisListType.X, op=OP.add)
                nc.sync.dma_start(st_in_b[:], stats[:])
                nc.gpsimd.collective_compute(
                    "AllReduce", OP.add, replica_groups=G8,
                    ins=[st_in_b[:].opt()], outs=[st_out_b[:].opt()])
                nc.sync.dma_start(stot[:], st_out_a[:])
                nc.sync.dma_start(stot_b[:], st_out_b[:])
                nc.vector.tensor_add(stot[:], stot[:], stot_b[:])

                # every sample's full conv is present on both pair cores,
                # so the 8-core sum double counts: divide by 2*B*L
                inv = 1.0 / (2.0 * B * L)
                mean = tl.tile([C, 1], F32)
                ex2 = tl.tile([C, 1], F32)
                var = tl.tile([C, 1], F32)
                tmp = tl.tile([C, 1], F32)
                nc.vector.tensor_scalar_mul(mean[:], stot[:, 0:1], inv)
                nc.vector.tensor_scalar_mul(ex2[:], stot[:, 1:2], inv)
                nc.vector.tensor_mul(tmp[:], mean[:], mean[:])
                nc.vector.tensor_sub(var[:], ex2[:], tmp[:])
                # invstd = exp(-0.5*ln(var+eps)) -- ln/exp stay in the
                # loaded table set (no sqrt-set reload on the tail)
                nc.vector.tensor_scalar_add(var[:], var[:], 1e-5)
                nc.scalar.activation(tmp[:], var[:], AF.Ln)
                nc.scalar.activation(tmp[:], tmp[:], AF.Exp, scale=-0.5)
                scal = tl.tile([C, 1], F32)
                shft = tl.tile([C, 1], F32)
                nc.vector.tensor_mul(scal[:], p_bng, tmp[:])
                nc.vector.tensor_mul(tmp[:], mean[:], scal[:])
                nc.vector.tensor_sub(shft[:], p_bnb, tmp[:])

                # bn + residual + leaky relu: out = prelu(conv*scal + res
                # + shft); conv*scal on ACT (per-partition scale), add on
                # DVE at 2x, prelu+shift on ACT straight to f32 out
                for lo in range(0, L, 1024):
                    hi = lo + 1024
                    bs = plf.tile([C, 1024], BF16, tag="bn")
                    nc.scalar.activation(bs[:], conv_sb[:, lo:hi],
                                         AF.Copy, scale=scal[:, 0:1])
                    nc.vector.tensor_add(bs[:], bs[:], res_sb[:, lo:hi])
                    ot = plf.tile([C, 1024], F32, tag="ot")
                    nc.scalar.activation(ot[:], bs[:],
                                         AF.Prelu, alpha=0.01,
                                         bias=shft[:, 0:1])
                    nc.sync.dma_start(out_d[:, lo:hi], ot[:])

    nc.compile()
    return nc


_NC = None


def _get_nc():
    global _NC
    if _NC is None:
        _NC = _build()
    return _NC


def _prep_in_maps(inp):
    inp = {k: np.asarray(v, dtype=np.float32) for k, v in inp.items()}
    x = inp["x"]  # (4, 64, 64, 64)
    # full 3x3 conv weights over both direction blocks, [in=128, 9*64]
    c3 = np.zeros((128, 9 * C), np.float32)
    for ky in range(3):
        for kx in range(3):
            c3[:, (ky * 3 + kx) * C:(ky * 3 + kx + 1) * C] = \
                inp["conv_w"][:, :, ky, kx].T
    maps = []
    for core in range(NCORE):
        b, d = core // 2, core % 2
        pre = "m1_" if d == 0 else "m2_"
        in_w = inp[pre + "in_w"]          # (256, 64)
        xproj_w = inp[pre + "xproj_w"]    # (36, 128)
        dt_w = inp[pre + "dt_w"]          # (128, 4)
        conv1_w = inp[pre + "conv_w"]     # (128, 4)

        x_loc = x[b].reshape(C, L)
        if d == 1:
            x_loc = x_loc[:, ::-1]

        bigproj = dt_w @ xproj_w[:DTR]    # (128, 128)

        blob_f = np.zeros((128, BF_COLS), np.float32)
        blob_f[:, 0] = inp[pre + "conv_b"]
        blob_f[:, 1] = inp[pre + "dt_b"]
        blob_f[:, 2:18] = -np.exp(inp[pre + "A_log"])
        blob_f[:, 18] = inp[pre + "D"]
        blob_f[:C, 19] = inp["conv_b"]
        blob_f[:C, 20] = inp["res_b"]
        blob_f[:C, 21] = inp["bn_gamma"]
        blob_f[:C, 22] = inp["bn_beta"]
        blob_f[:, 23] = -inp[pre + "conv_b"]

        blob_h = np.zeros((128, BH_COLS), np.float32)
        blob_h[:, OFF_C3W:OFF_OWT] = c3
        blob_h[:, OFF_OWT:OFF_BIG] = inp[pre + "out_w"].T
        blob_h[:, OFF_BIG:OFF_BCW] = bigproj.T
        blob_h[:, OFF_BCW:OFF_WK] = xproj_w[DTR:].T
        # fused in-projection + depthwise causal conv:
        # W_k[ch_x, di] = in_w[di, ch_x] * conv1_w[di, k]
        xi_w = in_w[:DI]                  # (128, 64)
        for k in range(DCONV):
            blob_h[:C, OFF_WK + 128 * k:OFF_WK + 128 * (k + 1)] = \
                (xi_w * conv1_w[:, k:k + 1]).T
        blob_h[:C, OFF_ZWT:OFF_ZWT + 128] = in_w[DI:].T
        blob_h[:C, OFF_RWT:OFF_RWT + C] = inp["res_w"].T
        m = {
            "x_loc": np.ascontiguousarray(x_loc).astype(ml_dtypes.bfloat16),
            "blob_f": blob_f,
            "blob_h": blob_h.astype(ml_dtypes.bfloat16),
        }
        maps.append(m)
    return maps


def _run(inputs, trace=False):
    nc = _get_nc()
    maps = _prep_in_maps(inputs)
    res = bass_utils.run_bass_kernel_spmd(
        nc, maps, core_ids=list(range(NCORE)), trace=trace)
    out = np.stack([res.results[2 * b]["out"].reshape(C, H, W)
                    for b in range(B)])
    return out, res


def kernel(**inputs) -> np.ndarray:
    out, _ = _run(inputs, trace=False)
    return out


# revision 48
# speedup vs baseline: 1.0727x; 1.0727x over previous
"""BiMamba block kernel for 8 Trainium2 NeuronCores.

Sharding: core = 2*sample + direction (4 samples x 2 scan directions).
Each core runs the full mamba for its (sample, direction).

The selective scan dominates on DVE (16 states x 4096 cols at ~2.1
ns/col is irreducible), so the kernel keeps the DVE queue dense and
hides everything else under it:

- All activations use only the natural_log_exp ACT table set: silu is
  computed as v*sigmoid(v) with sigmoid(v) = exp(-softplus(-v)), so no
  table reload ever interrupts the exp stream of the scan.
- Front: fused in-proj + causal conv matmuls with one-chunk lookahead
  on the PE queue (chunk c+1's input matmuls are emitted before chunk
  c's xc-dependent ones, so the PE never idles on a silu).
- Scan: 3 segments (2048/1536/512).  The DVE state loop is software
  pipelined (scan_s | dbx_{s+1} | g_s) so no two adjacent DVE ops are
  dependent.  B/C rows arrive via DRAM partition-broadcast DMAs issued
  6 states ahead.
- Exchange: after each segment the out-projection rows are AllGathered
  with the pair core (rank order = [dir0; dir1], matching the
  reference's un-unflipped y2 concat); the 3x3 conv runs locally.
  A dummy 8-core AllReduce early in the kernel absorbs core launch
  skew (~45-60 us) so tail collectives don't pay it.
- Conv is cut into row-pieces such that only rows 55-63 depend on the
  last exchange; BN stats for pieces 0-6 AllReduce early.  invstd via
  ln/exp (no sqrt table load).
"""
import os
import sys

for _p in ("/opt/trn_rl_repo", "/root/.axon_site/_ro/trn_rl_repo"):
    if os.path.isdir(_p):
        if _p not in sys.path:
            sys.path.insert(0, _p)
        break

import ml_dtypes
import numpy as np

# The agent image's antenv lacks axon_hooks; inject it so trace=True can
# capture NTFF profiles (used by test.py for HW timing, not for grading).
try:
    import antenv.axon_hooks  # noqa: F401
except ImportError:
    try:
        import types as _types

        from trn_agent_boot.trn_boot import _ntff_profile_via_ctypes

        _hook = _ntff_profile_via_ctypes("/opt/axon/libaxon_pjrt.so")
        _m = _types.ModuleType("antenv.axon_hooks")
        _m.get_axon_ntff_profile_hook = lambda: _hook
        _m.set_axon_ntff_profile_hook = lambda h: None
        sys.modules["antenv.axon_hooks"] = _m
    except Exception:
        pass

import concourse.bass as bass
import concourse.mybir as mybir
from concourse import bacc
from concourse import bass_utils
from concourse.masks import make_identity
from concourse.tile import TileContext

F32 = mybir.dt.float32
BF16 = mybir.dt.bfloat16
AF = mybir.ActivationFunctionType
OP = mybir.AluOpType

B, C, H, W = 4, 64, 64, 64
L = H * W          # 4096
DI = 128           # d_inner
DS = 16            # d_state
DTR = 4            # dt_rank
DCONV = 4
NCORE = 8
CH = 512           # matmul free-dim chunk
NCH = L // CH      # 8
RPC = CH // W      # output rows per chunk (8)

SEGS = ((0, 2048), (2048, 3584), (3584, 4096))
SEG_CHUNKS = ((0, 1, 2, 3), (4, 5, 6), (7,))
NSEG = len(SEGS)
WAVES = ((0, (0, 1, 2, 3)), (1, (4, 5, 6)), (2, (7,)))
# conv pieces (row ranges): pieces 0-6 need only waves 0-1; pieces 7-8
# (rows 55-63, PSUM-bank-sized) are the only ones gated on wave 2
CONV_PIECES = ((0, 8), (8, 16), (16, 24), (24, 32), (32, 40), (40, 48),
               (48, 55), (55, 63), (63, 64))
NPIECE = len(CONV_PIECES)

# blob_h layout (bf16): c3w | owT | bigT | bcwT | wk0..3 | zwT | rwT
OFF_C3W = 0
OFF_OWT = OFF_C3W + 9 * C
OFF_BIG = OFF_OWT + C
OFF_BCW = OFF_BIG + 128
OFF_WK = OFF_BCW + 32
OFF_ZWT = OFF_WK + 4 * 128
OFF_RWT = OFF_ZWT + 128
BH_COLS = OFF_RWT + C
BF_COLS = 32


def _build():
    nc = bacc.Bacc(target_bir_lowering=False, debug=False, num_devices=NCORE)

    def din(name, shape, dtype=F32):
        return nc.dram_tensor(name, shape, dtype, kind="ExternalInput")

    F32R = mybir.dt.float32r
    x_loc = din("x_loc", [C, L], BF16)
    blob_f = din("blob_f", [128, BF_COLS], F32)
    blob_h = din("blob_h", [128, BH_COLS], BF16)

    out_d = nc.dram_tensor("out", [C, L], F32, kind="ExternalOutput")

    with TileContext(nc) as tc:
        with tc.tile_pool(name="pers", bufs=1) as pers:
            # ---- params arrive as two packed blobs ----
            p_bf = pers.tile([128, BF_COLS], F32)
            p_bh = pers.tile([128, BH_COLS], BF16)
            nc.sync.dma_start(p_bf[:], blob_f[:])
            nc.sync.dma_start(p_bh[:], blob_h[:])
            p_c1b = p_bf[:, 0:1]
            p_dtb = p_bf[:, 1:2]
            p_A = p_bf[:, 2:18]
            p_D = p_bf[:, 18:19]
            p_c3b = p_bf[:C, 19:20]
            p_rb = p_bf[:C, 20:21]
            p_bng = p_bf[:C, 21:22]
            p_bnb = p_bf[:C, 22:23]
            p_nc1b = p_bf[:, 23:24]
            p_c3w = p_bh[:, OFF_C3W:OFF_OWT]
            p_owT = p_bh[:, OFF_OWT:OFF_BIG]
            p_bigT = p_bh[:, OFF_BIG:OFF_BCW]
            p_bcwT = p_bh[:, OFF_BCW:OFF_WK]
            p_wk = [p_bh[:, OFF_WK + 128 * k:OFF_WK + 128 * (k + 1)]
                    for k in range(DCONV)]
            p_zwT = p_bh[:, OFF_ZWT:OFF_ZWT + 128]
            p_rwT = p_bh[:, OFF_RWT:OFF_RWT + C]

            ident = pers.tile([128, 128], F32)
            make_identity(nc, ident[:])
            ident_g = pers.tile([128, 128], BF16)
            nc.vector.tensor_copy(ident_g[:], ident[:])

            # DRAM staging for B/C rows (DMA partition-broadcast needs a
            # DRAM source)
            bc_dram = nc.dram_tensor("bc_stage", [2 * DS, L], BF16)

            x_pad = pers.tile([64, 3 + L], BF16)
            nc.gpsimd.memset(x_pad[:, 0:3], 0.0)
            # split load so the front chunks start without waiting for
            # the full x (chunk 3's last tap reads through x col 2048)
            nc.sync.dma_start(x_pad[:, 3:3 + 2064], x_loc[:, 0:2064])
            nc.sync.dma_start(x_pad[:, 3 + 2064:3 + L], x_loc[:, 2064:L])

            with tc.tile_pool(name="smid", bufs=1) as smid, \
                 tc.tile_pool(name="ps", bufs=4, space="PSUM") as psp, \
                 tc.tile_pool(name="psy", bufs=4, space="PSUM") as psy, \
                 tc.tile_pool(name="sl_e", bufs=2) as plex, \
                 tc.tile_pool(name="sl_a", bufs=3) as pla, \
                 tc.tile_pool(name="sl_b", bufs=5) as plb, \
                 tc.tile_pool(name="sl_x", bufs=3) as plx, \
                 tc.tile_pool(name="sl_h", bufs=3) as plh, \
                 tc.tile_pool(name="sl_c", bufs=5) as plc, \
                 tc.tile_pool(name="sl_g", bufs=3) as plg, \
                 tc.tile_pool(name="sl_f", bufs=2) as plf, \
                 tc.tile_pool(name="dram", bufs=1, space="DRAM") as dr:
                z_sil = smid.tile([DI, L], BF16)
                dtv = smid.tile([DI, L], BF16)
                dtxc = smid.tile([DI, L], BF16)
                xcd = smid.tile([DI, L], BF16)
                xc = smid.tile([DI, L], BF16)
                carry = smid.tile([DI, DS], F32)

                ympad = smid.tile([128, H + 2, W + 2], BF16)
                nc.gpsimd.memset(ympad[:], 0.0)
                # persistent SBUF copy of the B/C rows: source for the
                # gpsimd partition-broadcasts (C) and the bc_dram staging
                bc_sb = smid.tile([2 * DS, L], BF16)
                # res rows 0-63, conv rows 64-127 share one tile
                rescv = smid.tile([128, L], BF16)
                res_sb = rescv[0:C]
                conv_sb = rescv[C:128]
                stats_m = smid.tile([C, NPIECE], F32)
                stats_v = smid.tile([C, NPIECE], F32)
                PAIRS = [[0, 1], [2, 3], [4, 5], [6, 7]]
                G8 = [[0, 1, 2, 3, 4, 5, 6, 7]]

                cc_ins, cc_outs = [], []
                for wi, (_, cvs) in enumerate(WAVES):
                    cc_ins.append(dr.tile([C, len(cvs) * CH], BF16,
                                          name=f"cci{wi}"))
                    cc_outs.append(dr.tile([128, len(cvs) * CH], BF16,
                                           name=f"cco{wi}"))
                st_in_a = dr.tile([C, 2], F32, name="st_in_a")
                st_in_b = dr.tile([C, 2], F32, name="st_in_b")
                sync_in = dr.tile([C, 2], F32, name="sync_in")
                st_out_a = nc.dram_tensor("st_out_a", [C, 2], F32,
                                          addr_space="Shared")
                st_out_b = nc.dram_tensor("st_out_b", [C, 2], F32,
                                          addr_space="Shared")
                sync_out = nc.dram_tensor("sync_out", [C, 2], F32,
                                          addr_space="Shared")

                def sigmoid_mul2(jobs):
                    """For each (dst, ps, bias, nbias) job: dst =
                    (v+b)*sigmoid(v+b), sigmoid via exp(-ln(1+exp(-(v+b))))
                    -- exp/ln only, one table set.  The jobs' ACT chains
                    are interleaved so no two adjacent ACT ops depend on
                    each other (hides the write-ack latency)."""
                    ts = []
                    for dst_sl, ps_t, bias, nbias in jobs:
                        t = plf.tile([DI, CH], BF16, tag="sg")
                        if nbias is not None:
                            nc.scalar.activation(t[:], ps_t[:DI], AF.Exp,
                                                 scale=-1.0, bias=nbias)
                        else:
                            nc.scalar.activation(t[:], ps_t[:DI], AF.Exp,
                                                 scale=-1.0)
                        ts.append(t)
                    for t in ts:
                        nc.scalar.activation(t[:], t[:], AF.Ln, bias=1.0)
                    for t in ts:
                        nc.scalar.activation(t[:], t[:], AF.Exp, scale=-1.0)
                    for t, (dst_sl, ps_t, bias, _) in zip(ts, jobs):
                        nc.vector.scalar_tensor_tensor(
                            dst_sl, ps_t[:DI],
                            bias if bias is not None else 0.0,
                            t[:], op0=OP.add, op1=OP.mult)

                def front_in(c):
                    """Input-dependent matmuls for chunk c (no xc dep)."""
                    ps = psp.tile([128, CH], F32, tag="ps", name=f"fi{c}")
                    for k in range(DCONV):
                        nc.tensor.matmul(ps[:DI], p_wk[k][:C],
                                         x_pad[:, c * CH + k:c * CH + k + CH],
                                         start=(k == 0), stop=(k == DCONV - 1))
                    ps2 = psp.tile([128, CH], F32, tag="ps", name=f"fz{c}")
                    nc.tensor.matmul(ps2[:DI], p_zwT[:C],
                                     x_pad[:, 3 + c * CH:3 + (c + 1) * CH],
                                     start=True, stop=True)
                    return ps, ps2

                def front_mid(c, ps, ps2):
                    """silus for chunk c (ACT+DVE)."""
                    sl = slice(c * CH, (c + 1) * CH)
                    sigmoid_mul2([(xc[:, sl], ps, p_c1b, p_nc1b),
                                  (z_sil[:, sl], ps2, None, None)])

                def front_out(c, with_ln):
                    """xc-dependent projections for chunk c."""
                    sl = slice(c * CH, (c + 1) * CH)
                    ps3 = psp.tile([128, CH], F32, tag="ps", name=f"fd{c}")
                    nc.tensor.matmul(ps3[:DI], p_bigT[:], xc[:, sl],
                                     start=True, stop=True)
                    nc.scalar.activation(dtv[:, sl], ps3[:DI], AF.Exp,
                                         bias=p_dtb)
                    ps4 = psp.tile([128, CH], F32, tag="ps", name=f"fb{c}")
                    nc.tensor.matmul(ps4[:2 * DS], p_bcwT[:], xc[:, sl],
                                     start=True, stop=True)
                    nc.scalar.copy(bc_sb[:, sl], ps4[:2 * DS])
                    nc.sync.dma_start(bc_dram[:, sl], bc_sb[:, sl])
                    if with_ln:
                        nc.scalar.activation(dtv[:, sl], dtv[:, sl], AF.Ln,
                                             bias=1.0)
                        nc.vector.tensor_mul(dtxc[:, sl], dtv[:, sl],
                                             xc[:, sl])
                        nc.scalar.activation(xcd[:, sl], xc[:, sl],
                                             AF.Copy, scale=p_D)

                def front_chunk(c, with_ln):
                    ps, ps2 = front_in(c)
                    front_mid(c, ps, ps2)
                    front_out(c, with_ln)

                def finish_front(cs):
                    hsl = slice(cs[0] * CH, (cs[-1] + 1) * CH)
                    nc.scalar.activation(dtv[:, hsl], dtv[:, hsl], AF.Ln,
                                         bias=1.0)
                    for c in cs:
                        sl = slice(c * CH, (c + 1) * CH)
                        nc.vector.tensor_mul(dtxc[:, sl], dtv[:, sl],
                                             xc[:, sl])
                        nc.scalar.activation(xcd[:, sl], xc[:, sl],
                                             AF.Copy, scale=p_D)

                def wave(wi):
                    """Out-projection + pair AllGather + ympad write +
                    residual for the wave's chunks.  AllGather output is
                    rank-ordered, so both cores get [dir0; dir1]."""
                    cvs = WAVES[wi][1]
                    stage = plex.tile([C, len(cvs) * CH], BF16,
                                      tag="stage", name=f"stage{wi}")
                    for j, cix in enumerate(cvs):
                        sl = slice(cix * CH, (cix + 1) * CH)
                        ssl = slice(j * CH, (j + 1) * CH)
                        yg = plf.tile([DI, CH], BF16, tag="yg")
                        nc.vector.tensor_mul(yg[:], y_ps[cix][:DI],
                                             z_sil[:, sl])
                        po = psp.tile([128, CH], F32, tag="ps",
                                      name=f"po{cix}")
                        nc.tensor.matmul(po[:C], p_owT[:], yg[:],
                                         start=True, stop=True)
                        nc.scalar.copy(stage[:, ssl], po[:C])
                        psr = psp.tile([128, CH], F32, tag="ps",
                                       name=f"rs{cix}")
                        nc.tensor.matmul(psr[:C], p_rwT[:C],
                                         x_pad[:, 3 + cix * CH:
                                               3 + (cix + 1) * CH],
                                         start=True, stop=True)
                        nc.scalar.activation(res_sb[:, sl], psr[:C],
                                             AF.Identity, bias=p_rb)
                    nc.sync.dma_start(cc_ins[wi][:], stage[:])
                    nc.gpsimd.collective_compute(
                        "AllGather", OP.bypass, replica_groups=PAIRS,
                        ins=[cc_ins[wi][:].opt()], outs=[cc_outs[wi][:].opt()])
                    r0 = cvs[0] * RPC
                    nrows = len(cvs) * RPC
                    nc.sync.dma_start(
                        ympad[:, 1 + r0:1 + r0 + nrows, 1:1 + W],
                        cc_outs[wi][:].rearrange("p (r w) -> p r w", w=W))

                def conv3_piece(i):
                    r0, r1 = CONV_PIECES[i]
                    nr = r1 - r0
                    ps = psp.tile([128, nr * W], F32, tag="ps",
                                  name=f"cv{i}")
                    ps3 = ps[:C].rearrange("p (r w) -> p r w", w=W)
                    n = 0
                    for ky in range(3):
                        for kx in range(3):
                            nc.tensor.matmul(
                                ps3[:],
                                p_c3w[:, (ky * 3 + kx) * C:
                                      (ky * 3 + kx + 1) * C],
                                ympad[:, r0 + ky:r0 + ky + nr, kx:kx + W],
                                start=(n == 0), stop=(n == 8))
                            n += 1
                    sl = slice(r0 * W, r1 * W)
                    flat = ps3.rearrange("p r w -> p (r w)")
                    nc.scalar.activation(conv_sb[:, sl], flat,
                                         AF.Identity, bias=p_c3b,
                                         accum_out=stats_m[:, i:i + 1])
                    sq = plf.tile([C, nr * W], BF16, tag="sq")
                    nc.scalar.activation(sq[:], conv_sb[:, sl],
                                         AF.Square,
                                         accum_out=stats_v[:, i:i + 1])

                stats_a = smid.tile([C, 2], F32)

                def stats_early():
                    nc.vector.tensor_reduce(stats_a[:, 0:1],
                                            stats_m[:, 0:7],
                                            axis=mybir.AxisListType.X,
                                            op=OP.add)
                    nc.vector.tensor_reduce(stats_a[:, 1:2],
                                            stats_v[:, 0:7],
                                            axis=mybir.AxisListType.X,
                                            op=OP.add)
                    nc.sync.dma_start(st_in_a[:], stats_a[:])
                    nc.gpsimd.collective_compute(
                        "AllReduce", OP.add, replica_groups=G8,
                        ins=[st_in_a[:].opt()], outs=[st_out_a[:].opt()])

                y_ps = {}
                bc_q = {}
                PREF = 4

                def bc_issue(q, s):
                    t0, t1 = SEGS[q]
                    SEG = t1 - t0
                    qsl = slice(t0, t1)
                    bbc = plb.tile([DI, SEG], BF16, tag="bbc",
                                   name=f"bbc{q}_{s}")
                    nc.sync.dma_start(
                        bbc[:],
                        bc_dram[s:s + 1, qsl].to_broadcast((DI, SEG)))
                    # C rows broadcast on the (otherwise idle) Pool engine
                    # to halve the DMA broadcast traffic; the source must
                    # sit at partition 0, so stage the 4KB row first
                    cst = plc.tile([1, SEG], BF16, tag="cst",
                                   name=f"cst{q}_{s}")
                    nc.sync.dma_start(cst[:], bc_dram[DS + s:DS + s + 1,
                                                      qsl])
                    cbc = plc.tile([DI, SEG], BF16, tag="cbc",
                                   name=f"cbc{q}_{s}")
                    nc.gpsimd.partition_broadcast(cbc[:], cst[:])
                    bc_q[(q, s)] = (bbc, cbc)

                def seg_prefetch(q):
                    for s in range(PREF):
                        bc_issue(q, s)

                def seg_scan(q, work):
                    t0, t1 = SEGS[q]
                    SEG = t1 - t0
                    qsl = slice(t0, t1)
                    for cix in SEG_CHUNKS[q]:
                        yp = psy.tile([128, CH], F32, tag="yps",
                                      name=f"y{cix}")
                        nc.tensor.matmul(yp[:DI], ident_g[:],
                                         xcd[:, cix * CH:(cix + 1) * CH],
                                         start=True, stop=False)
                        y_ps[cix] = yp
                    # software-pipelined state loop: no two adjacent DVE
                    # ops are data-dependent (scan_s | dbx_{s+1} | g_s)
                    da0 = pla.tile([DI, SEG], BF16, tag="da")
                    nc.scalar.activation(da0[:], dtv[:, qsl], AF.Exp,
                                         scale=p_A[:, 0:1])
                    bbc0, _ = bc_q[(q, 0)]
                    dbx_n = plx.tile([DI, SEG], BF16, tag="dbx")
                    nc.vector.tensor_mul(dbx_n[:], dtxc[:, qsl], bbc0[:])
                    da_n = da0
                    for s in range(DS):
                        da, dbx = da_n, dbx_n
                        _, cbc = bc_q.pop((q, s))
                        if s + 1 < DS:
                            da_n = pla.tile([DI, SEG], BF16, tag="da")
                            nc.scalar.activation(da_n[:], dtv[:, qsl],
                                                 AF.Exp,
                                                 scale=p_A[:, s + 1:s + 2])
                        h = plh.tile([DI, SEG], BF16, tag="h")
                        init = 0.0 if q == 0 else carry[:, s:s + 1]
                        nc.vector.tensor_tensor_scan(h[:], da[:], dbx[:],
                                                     init, op0=OP.mult,
                                                     op1=OP.add)
                        if s + 1 < DS:
                            bbc_n, _ = bc_q[(q, s + 1)]
                            dbx_n = plx.tile([DI, SEG], BF16, tag="dbx")
                            nc.vector.tensor_mul(dbx_n[:], dtxc[:, qsl],
                                                 bbc_n[:])
                        g = plg.tile([DI, SEG], BF16, tag="g")
                        nc.vector.tensor_mul(g[:], h[:], cbc[:])
                        if q < NSEG - 1:
                            nc.vector.tensor_copy(carry[:, s:s + 1],
                                                  h[:, SEG - 1:SEG])
                        for j, cix in enumerate(SEG_CHUNKS[q]):
                            nc.tensor.matmul(
                                y_ps[cix][:DI], ident_g[:],
                                g[:, j * CH:(j + 1) * CH],
                                start=False, stop=(s == DS - 1))
                        if s + PREF < DS:
                            bc_issue(q, s + PREF)
                        if s in work:
                            work[s]()

                # =========== emission ===========
                # front with one-chunk PE lookahead: chunk c+1's input
                # matmuls are emitted before chunk c's xc-dependent ones
                pends = {0: front_in(0)}
                # dummy 8-core sync: absorbs core launch skew while the
                # front runs, so tail collectives don't pay it
                nc.gpsimd.memset(stats_a[:], 0.0)
                nc.sync.dma_start(sync_in[:], stats_a[:])
                nc.gpsimd.collective_compute(
                    "AllReduce", OP.add, replica_groups=G8,
                    ins=[sync_in[:].opt()], outs=[sync_out[:].opt()])
                for c in range(4):
                    if c + 1 < 4:
                        pends[c + 1] = front_in(c + 1)
                    front_mid(c, *pends.pop(c))
                    front_out(c, False)
                finish_front((0, 1, 2, 3))

                seg_prefetch(0)
                seg_scan(0, {2: lambda: front_chunk(4, True),
                             5: lambda: front_chunk(5, True),
                             8: lambda: front_chunk(6, True),
                             11: lambda: front_chunk(7, True),
                             13: lambda: seg_prefetch(1)})
                wave(0)
                seg_scan(1, {6: lambda: conv3_piece(0),
                             9: lambda: conv3_piece(1),
                             12: lambda: conv3_piece(2),
                             13: lambda: seg_prefetch(2)})
                wave(1)
                seg_scan(2, {})
                wave(2)
                # conv pieces 3-6 hide the last exchange's latency on PE
                conv3_piece(3)
                conv3_piece(4)
                conv3_piece(5)
                conv3_piece(6)
                stats_early()
                conv3_piece(7)
                conv3_piece(8)

                # ---- batch stats AllReduces + BN + residual + leaky ----
                tl = smid
                stot = tl.tile([C, 2], F32)
                stot_b = tl.tile([C, 2], F32)
                stats = tl.tile([C, 2], F32)
                nc.vector.tensor_reduce(stats[:, 0:1], stats_m[:, 7:9],
                                        axis=mybir.AxisListType.X, op=OP.add)
                nc.vector.tensor_reduce(stats[:, 1:2], stats_v[:, 7:9],
                                        axis=mybir.AxisListType.X, op=OP.add)
                nc.sync.dma_start(st_in_b[:], stats[:])
                nc.gpsimd.collective_compute(
                    "AllReduce", OP.add, replica_groups=G8,
                    ins=[st_in_b[:].opt()], outs=[st_out_b[:].opt()])
                nc.sync.dma_start(stot[:], st_out_a[:])
                nc.sync.dma_start(stot_b[:], st_out_b[:])
                nc.vector.tensor_add(stot[:], stot[:], stot_b[:])

                # every sample's full conv is present on both pair cores,
                # so the 8-core sum double counts: divide by 2*B*L
                inv = 1.0 / (2.0 * B * L)
                mean = tl.tile([C, 1], F32)
                ex2 = tl.tile([C, 1], F32)
                var = tl.tile([C, 1], F32)
                tmp = tl.tile([C, 1], F32)
                nc.vector.tensor_scalar_mul(mean[:], stot[:, 0:1], inv)
                nc.vector.tensor_scalar_mul(ex2[:], stot[:, 1:2], inv)
                nc.vector.tensor_mul(tmp[:], mean[:], mean[:])
                nc.vector.tensor_sub(var[:], ex2[:], tmp[:])
                # invstd = exp(-0.5*ln(var+eps)) -- ln/exp stay in the
                # loaded table set (no sqrt-set reload on the tail)
                nc.vector.tensor_scalar_add(var[:], var[:], 1e-5)
                nc.scalar.activation(tmp[:], var[:], AF.Ln)
                nc.scalar.activation(tmp[:], tmp[:], AF.Exp, scale=-0.5)
                scal = tl.tile([C, 1], F32)
                shft = tl.tile([C, 1], F32)
                nc.vector.tensor_mul(scal[:], p_bng, tmp[:])
                nc.vector.tensor_mul(tmp[:], mean[:], scal[:])
                nc.vector.tensor_sub(shft[:], p_bnb, tmp[:])

                # bn + residual + leaky relu: out = prelu(conv*scal + res
                # + shft); conv*scal on ACT (per-partition scale), add on
                # DVE at 2x, prelu+shift on ACT straight to f32 out
                for lo in range(0, L, 512):
                    hi = lo + 512
                    bs = plf.tile([C, 512], BF16, tag="bn")
                    nc.scalar.activation(bs[:], conv_sb[:, lo:hi],
                                         AF.Copy, scale=scal[:, 0:1])
                    nc.vector.tensor_add(bs[:], bs[:], res_sb[:, lo:hi])
                    ot = plf.tile([C, 512], F32, tag="ot")
                    nc.scalar.activation(ot[:], bs[:],
                                         AF.Prelu, alpha=0.01,
                                         bias=shft[:, 0:1])
                    nc.sync.dma_start(out_d[:, lo:hi], ot[:])

    nc.compile()
    return nc


_NC = None


def _get_nc():
    global _NC
    if _NC is None:
        _NC = _build()
    return _NC


def _prep_in_maps(inp):
    inp = {k: np.asarray(v, dtype=np.float32) for k, v in inp.items()}
    x = inp["x"]  # (4, 64, 64, 64)
    # full 3x3 conv weights over both direction blocks, [in=128, 9*64]
    c3 = np.zeros((128, 9 * C), np.float32)
    for ky in range(3):
        for kx in range(3):
            c3[:, (ky * 3 + kx) * C:(ky * 3 + kx + 1) * C] = \
                inp["conv_w"][:, :, ky, kx].T
    maps = []
    for core in range(NCORE):
        b, d = core // 2, core % 2
        pre = "m1_" if d == 0 else "m2_"
        in_w = inp[pre + "in_w"]          # (256, 64)
        xproj_w = inp[pre + "xproj_w"]    # (36, 128)
        dt_w = inp[pre + "dt_w"]          # (128, 4)
        conv1_w = inp[pre + "conv_w"]     # (128, 4)

        x_loc = x[b].reshape(C, L)
        if d == 1:
            x_loc = x_loc[:, ::-1]

        bigproj = dt_w @ xproj_w[:DTR]    # (128, 128)

        blob_f = np.zeros((128, BF_COLS), np.float32)
        blob_f[:, 0] = inp[pre + "conv_b"]
        blob_f[:, 1] = inp[pre + "dt_b"]
        blob_f[:, 2:18] = -np.exp(inp[pre + "A_log"])
        blob_f[:, 18] = inp[pre + "D"]
        blob_f[:C, 19] = inp["conv_b"]
        blob_f[:C, 20] = inp["res_b"]
        blob_f[:C, 21] = inp["bn_gamma"]
        blob_f[:C, 22] = inp["bn_beta"]
        blob_f[:, 23] = -inp[pre + "conv_b"]

        blob_h = np.zeros((128, BH_COLS), np.float32)
        blob_h[:, OFF_C3W:OFF_OWT] = c3
        blob_h[:, OFF_OWT:OFF_BIG] = inp[pre + "out_w"].T
        blob_h[:, OFF_BIG:OFF_BCW] = bigproj.T
        blob_h[:, OFF_BCW:OFF_WK] = xproj_w[DTR:].T
        # fused in-projection + depthwise causal conv:
        # W_k[ch_x, di] = in_w[di, ch_x] * conv1_w[di, k]
        xi_w = in_w[:DI]                  # (128, 64)
        for k in range(DCONV):
            blob_h[:C, OFF_WK + 128 * k:OFF_WK + 128 * (k + 1)] = \
                (xi_w * conv1_w[:, k:k + 1]).T
        blob_h[:C, OFF_ZWT:OFF_ZWT + 128] = in_w[DI:].T
        blob_h[:C, OFF_RWT:OFF_RWT + C] = inp["res_w"].T
        m = {
            "x_loc": np.ascontiguousarray(x_loc).astype(ml_dtypes.bfloat16),
            "blob_f": blob_f,
            "blob_h": blob_h.astype(ml_dtypes.bfloat16),
        }
        maps.append(m)
    return maps


def _run(inputs, trace=False):
    nc = _get_nc()
    maps = _prep_in_maps(inputs)
    res = bass_utils.run_bass_kernel_spmd(
        nc, maps, core_ids=list(range(NCORE)), trace=trace)
    out = np.stack([res.results[2 * b]["out"].reshape(C, H, W)
                    for b in range(B)])
    return out, res


def kernel(**inputs) -> np.ndarray:
    out, _ = _run(inputs, trace=False)
    return out


# revision 49
# speedup vs baseline: 1.1680x; 1.0888x over previous
"""BiMamba block kernel for 8 Trainium2 NeuronCores.

Sharding: core = 2*sample + direction (4 samples x 2 scan directions).
Each core runs the full mamba for its (sample, direction).

The selective scan dominates on DVE (16 states x 4096 cols at ~2.1
ns/col, SBUF-bandwidth limited); the kernel keeps the DVE queue dense
and hides everything else under it:

- Front: fused in-proj + causal depthwise conv (4 accumulating
  matmuls, host-folded weights); silu in tanh form (one table set with
  exp), all 8 input/gate matmuls emitted before any xc-dependent
  matmul so the PE never idles on a silu; chunks 4-7 run between the
  first segment's scan states.
- Scan: 3 segments (2048/1536/512 cols).  Per state: ACT exp -> DVE
  dbx mul -> DVE tensor_tensor_scan -> DVE C mul -> PE PSUM
  accumulate.  B/C rows arrive via DRAM partition-broadcast DMAs
  issued 5 states ahead.
- Exchange: after each segment the out-projection rows are AllGathered
  with the pair core (rank order = [dir0; dir1], matching the
  reference's un-unflipped y2 concat); the full 3x3 conv then runs
  locally -- no collective after the conv.  A dummy 8-core AllReduce
  early in the kernel absorbs core launch skew (~45-60 us).
- Conv is cut into row-pieces such that only rows 55-63 depend on the
  last exchange; BN stats for pieces 0-6 AllReduce early, and invstd
  uses ln/exp so no ACT table load lands on the tail.
"""
import os
import sys

for _p in ("/opt/trn_rl_repo", "/root/.axon_site/_ro/trn_rl_repo"):
    if os.path.isdir(_p):
        if _p not in sys.path:
            sys.path.insert(0, _p)
        break

import ml_dtypes
import numpy as np

# The agent image's antenv lacks axon_hooks; inject it so trace=True can
# capture NTFF profiles (used by test.py for HW timing, not for grading).
try:
    import antenv.axon_hooks  # noqa: F401
except ImportError:
    try:
        import types as _types

        from trn_agent_boot.trn_boot import _ntff_profile_via_ctypes

        _hook = _ntff_profile_via_ctypes("/opt/axon/libaxon_pjrt.so")
        _m = _types.ModuleType("antenv.axon_hooks")
        _m.get_axon_ntff_profile_hook = lambda: _hook
        _m.set_axon_ntff_profile_hook = lambda h: None
        sys.modules["antenv.axon_hooks"] = _m
    except Exception:
        pass

import concourse.bass as bass
import concourse.mybir as mybir
from concourse import bacc
from concourse import bass_utils
from concourse.masks import make_identity
from concourse.tile import TileContext

F32 = mybir.dt.float32
BF16 = mybir.dt.bfloat16
AF = mybir.ActivationFunctionType
OP = mybir.AluOpType

B, C, H, W = 4, 64, 64, 64
L = H * W          # 4096
DI = 128           # d_inner
DS = 16            # d_state
DTR = 4            # dt_rank
DCONV = 4
NCORE = 8
CH = 512           # matmul free-dim chunk
NCH = L // CH      # 8
RPC = CH // W      # output rows per chunk (8)

SEGS = ((0, 2048), (2048, 3584), (3584, 4096))
SEG_CHUNKS = ((0, 1, 2, 3), (4, 5, 6), (7,))
NSEG = len(SEGS)
WAVES = ((0, (0, 1, 2, 3)), (1, (4, 5, 6)), (2, (7,)))
# conv pieces (row ranges): pieces 0-6 need only waves 0-1; pieces 7-8
# (rows 55-63, PSUM-bank-sized) are the only ones gated on wave 2
CONV_PIECES = ((0, 8), (8, 16), (16, 24), (24, 32), (32, 40), (40, 48),
               (48, 55), (55, 63), (63, 64))
NPIECE = len(CONV_PIECES)

# blob_h layout (bf16): c3w | owT | bigT | bcwT | wk0..3 | zwT | rwT
OFF_C3W = 0
OFF_OWT = OFF_C3W + 9 * C
OFF_BIG = OFF_OWT + C
OFF_BCW = OFF_BIG + 128
OFF_WK = OFF_BCW + 32
OFF_ZWT = OFF_WK + 4 * 128
OFF_RWT = OFF_ZWT + 128
BH_COLS = OFF_RWT + C
BF_COLS = 32


def _build():
    nc = bacc.Bacc(target_bir_lowering=False, debug=False, num_devices=NCORE)

    def din(name, shape, dtype=F32):
        return nc.dram_tensor(name, shape, dtype, kind="ExternalInput")

    x_loc = din("x_loc", [C, L], BF16)
    blob_f = din("blob_f", [128, BF_COLS], F32)
    blob_h = din("blob_h", [128, BH_COLS], BF16)

    out_d = nc.dram_tensor("out", [C, L], F32, kind="ExternalOutput")

    with TileContext(nc) as tc:
        with tc.tile_pool(name="pers", bufs=1) as pers:
            # ---- params arrive as two packed blobs ----
            p_bf = pers.tile([128, BF_COLS], F32)
            p_bh = pers.tile([128, BH_COLS], BF16)
            nc.sync.dma_start(p_bf[:], blob_f[:])
            nc.sync.dma_start(p_bh[:], blob_h[:])
            p_c1b = p_bf[:, 0:1]
            p_dtb = p_bf[:, 1:2]
            p_A = p_bf[:, 2:18]
            p_D = p_bf[:, 18:19]
            p_c3b = p_bf[:C, 19:20]
            p_rb = p_bf[:C, 20:21]
            p_bng = p_bf[:C, 21:22]
            p_bnb = p_bf[:C, 22:23]
            p_c1bh = p_bf[:, 23:24]   # 0.5 * conv1 bias
            p_c3w = p_bh[:, OFF_C3W:OFF_OWT]
            p_owT = p_bh[:, OFF_OWT:OFF_BIG]
            p_bigT = p_bh[:, OFF_BIG:OFF_BCW]
            p_bcwT = p_bh[:, OFF_BCW:OFF_WK]
            p_wk = [p_bh[:, OFF_WK + 128 * k:OFF_WK + 128 * (k + 1)]
                    for k in range(DCONV)]
            p_zwT = p_bh[:, OFF_ZWT:OFF_ZWT + 128]
            p_rwT = p_bh[:, OFF_RWT:OFF_RWT + C]

            ident = pers.tile([128, 128], F32)
            make_identity(nc, ident[:])
            ident_g = pers.tile([128, 128], BF16)
            nc.vector.tensor_copy(ident_g[:], ident[:])

            # DRAM staging for B/C rows (DMA partition-broadcast needs a
            # DRAM source)
            bc_dram = nc.dram_tensor("bc_stage", [2 * DS, L], BF16)

            x_pad = pers.tile([64, 3 + L], BF16)
            nc.gpsimd.memset(x_pad[:, 0:3], 0.0)
            # split load so the front chunks start without waiting for
            # the full x (chunk 3's last tap reads through x col 2048)
            nc.sync.dma_start(x_pad[:, 3:3 + 2064], x_loc[:, 0:2064])
            nc.sync.dma_start(x_pad[:, 3 + 2064:3 + L], x_loc[:, 2064:L])

            with tc.tile_pool(name="smid", bufs=1) as smid, \
                 tc.tile_pool(name="ps", bufs=4, space="PSUM") as psp, \
                 tc.tile_pool(name="psy", bufs=4, space="PSUM") as psy, \
                 tc.tile_pool(name="sl_e", bufs=2) as plex, \
                 tc.tile_pool(name="sl_a", bufs=3) as pla, \
                 tc.tile_pool(name="sl_b", bufs=5) as plb, \
                 tc.tile_pool(name="sl_x", bufs=3) as plx, \
                 tc.tile_pool(name="sl_h", bufs=3) as plh, \
                 tc.tile_pool(name="sl_c", bufs=5) as plc, \
                 tc.tile_pool(name="sl_g", bufs=3) as plg, \
                 tc.tile_pool(name="sl_f", bufs=2) as plf, \
                 tc.tile_pool(name="dram", bufs=1, space="DRAM") as dr:
                z_sil = smid.tile([DI, L], BF16)
                dtv = smid.tile([DI, L], BF16)
                dtxc = smid.tile([DI, L], BF16)
                xcd = smid.tile([DI, L], BF16)
                xc = smid.tile([DI, L], BF16)
                carry = smid.tile([DI, DS], F32)

                ympad = smid.tile([128, H + 2, W + 2], BF16)
                nc.gpsimd.memset(ympad[:], 0.0)
                # res rows 0-63, conv rows 64-127 share one tile
                rescv = smid.tile([128, L], BF16)
                res_sb = rescv[0:C]
                conv_sb = rescv[C:128]
                stats_m = smid.tile([C, NPIECE], F32)
                stats_v = smid.tile([C, NPIECE], F32)
                PAIRS = [[0, 1], [2, 3], [4, 5], [6, 7]]
                G8 = [[0, 1, 2, 3, 4, 5, 6, 7]]

                cc_ins, cc_outs = [], []
                for wi, (_, cvs) in enumerate(WAVES):
                    cc_ins.append(dr.tile([C, len(cvs) * CH], BF16,
                                          name=f"cci{wi}"))
                    cc_outs.append(dr.tile([128, len(cvs) * CH], BF16,
                                           name=f"cco{wi}"))
                st_in_a = dr.tile([C, 2], F32, name="st_in_a")
                st_in_b = dr.tile([C, 2], F32, name="st_in_b")
                sync_in = dr.tile([C, 2], F32, name="sync_in")
                st_out_a = nc.dram_tensor("st_out_a", [C, 2], F32,
                                          addr_space="Shared")
                st_out_b = nc.dram_tensor("st_out_b", [C, 2], F32,
                                          addr_space="Shared")
                sync_out = nc.dram_tensor("sync_out", [C, 2], F32,
                                          addr_space="Shared")

                def front_in(c):
                    """Input-dependent matmuls for chunk c (no xc dep)."""
                    ps = psp.tile([128, CH], F32, tag="ps", name=f"fi{c}")
                    for k in range(DCONV):
                        nc.tensor.matmul(ps[:DI], p_wk[k][:C],
                                         x_pad[:, c * CH + k:c * CH + k + CH],
                                         start=(k == 0), stop=(k == DCONV - 1))
                    ps2 = psp.tile([128, CH], F32, tag="ps", name=f"fz{c}")
                    nc.tensor.matmul(ps2[:DI], p_zwT[:C],
                                     x_pad[:, 3 + c * CH:3 + (c + 1) * CH],
                                     start=True, stop=True)
                    return ps, ps2

                def front_mid(c, ps, ps2):
                    """tanh-form silus for chunk c: xc and z_sil come out
                    scaled by 2; the 0.5/0.25 factors are folded into the
                    downstream weights on the host."""
                    sl = slice(c * CH, (c + 1) * CH)
                    th = plf.tile([DI, CH], BF16, tag="th")
                    nc.scalar.activation(th[:], ps[:DI], AF.Tanh,
                                         scale=0.5, bias=p_c1bh)
                    raw = plf.tile([DI, CH], BF16, tag="raw")
                    nc.scalar.activation(raw[:], ps[:DI], AF.Identity,
                                         bias=p_c1b)
                    nc.vector.scalar_tensor_tensor(
                        xc[:, sl], th[:], 1.0, raw[:],
                        op0=OP.add, op1=OP.mult)
                    th2 = plf.tile([DI, CH], BF16, tag="th")
                    nc.scalar.activation(th2[:], ps2[:DI], AF.Tanh,
                                         scale=0.5)
                    raw2 = plf.tile([DI, CH], BF16, tag="raw")
                    nc.scalar.copy(raw2[:], ps2[:DI])
                    nc.vector.scalar_tensor_tensor(
                        z_sil[:, sl], th2[:], 1.0, raw2[:],
                        op0=OP.add, op1=OP.mult)

                def front_out(c):
                    """xc-dependent projections for chunk c."""
                    sl = slice(c * CH, (c + 1) * CH)
                    ps3 = psp.tile([128, CH], F32, tag="ps", name=f"fd{c}")
                    nc.tensor.matmul(ps3[:DI], p_bigT[:], xc[:, sl],
                                     start=True, stop=True)
                    nc.scalar.activation(dtv[:, sl], ps3[:DI], AF.Exp,
                                         bias=p_dtb)
                    ps4 = psp.tile([128, CH], F32, tag="ps", name=f"fb{c}")
                    nc.tensor.matmul(ps4[:2 * DS], p_bcwT[:], xc[:, sl],
                                     start=True, stop=True)
                    bch = plb.tile([2 * DS, CH], BF16, tag="bch")
                    nc.scalar.copy(bch[:], ps4[:2 * DS])
                    nc.sync.dma_start(bc_dram[:, sl], bch[:])

                def finish_front(cs):
                    """softplus (batched Ln) + dtxc + xcd for chunks cs."""
                    hsl = slice(cs[0] * CH, (cs[-1] + 1) * CH)
                    nc.scalar.activation(dtv[:, hsl], dtv[:, hsl], AF.Ln,
                                         bias=1.0)
                    for c in cs:
                        sl = slice(c * CH, (c + 1) * CH)
                        nc.vector.tensor_mul(dtxc[:, sl], dtv[:, sl],
                                             xc[:, sl])
                        nc.scalar.activation(xcd[:, sl], xc[:, sl],
                                             AF.Copy, scale=p_D)

                def front_chunk(c, ln):
                    ps, ps2 = front_in(c)
                    front_mid(c, ps, ps2)
                    front_out(c)
                    if ln:
                        finish_front(ln)

                def wave(wi):
                    """Out-projection + pair AllGather + ympad write +
                    residual for the wave's chunks.  AllGather output is
                    rank-ordered, so both cores get [dir0; dir1]."""
                    cvs = WAVES[wi][1]
                    stage = plex.tile([C, len(cvs) * CH], BF16,
                                      tag="stage", name=f"stage{wi}")
                    for j, cix in enumerate(cvs):
                        sl = slice(cix * CH, (cix + 1) * CH)
                        ssl = slice(j * CH, (j + 1) * CH)
                        yg = plf.tile([DI, CH], BF16, tag="yg")
                        nc.vector.tensor_mul(yg[:], y_ps[cix][:DI],
                                             z_sil[:, sl])
                        po = psp.tile([128, CH], F32, tag="ps",
                                      name=f"po{cix}")
                        nc.tensor.matmul(po[:C], p_owT[:], yg[:],
                                         start=True, stop=True)
                        nc.scalar.copy(stage[:, ssl], po[:C])
                        psr = psp.tile([128, CH], F32, tag="ps",
                                       name=f"rs{cix}")
                        nc.tensor.matmul(psr[:C], p_rwT[:C],
                                         x_pad[:, 3 + cix * CH:
                                               3 + (cix + 1) * CH],
                                         start=True, stop=True)
                        nc.scalar.activation(res_sb[:, sl], psr[:C],
                                             AF.Identity, bias=p_rb)
                    nc.sync.dma_start(cc_ins[wi][:], stage[:])
                    nc.gpsimd.collective_compute(
                        "AllGather", OP.bypass, replica_groups=PAIRS,
                        ins=[cc_ins[wi][:].opt()], outs=[cc_outs[wi][:].opt()])
                    r0 = cvs[0] * RPC
                    nrows = len(cvs) * RPC
                    nc.sync.dma_start(
                        ympad[:, 1 + r0:1 + r0 + nrows, 1:1 + W],
                        cc_outs[wi][:].rearrange("p (r w) -> p r w", w=W))

                def conv3_piece(i):
                    r0, r1 = CONV_PIECES[i]
                    nr = r1 - r0
                    ps = psp.tile([128, nr * W], F32, tag="ps",
                                  name=f"cv{i}")
                    ps3 = ps[:C].rearrange("p (r w) -> p r w", w=W)
                    n = 0
                    for ky in range(3):
                        for kx in range(3):
                            nc.tensor.matmul(
                                ps3[:],
                                p_c3w[:, (ky * 3 + kx) * C:
                                      (ky * 3 + kx + 1) * C],
                                ympad[:, r0 + ky:r0 + ky + nr, kx:kx + W],
                                start=(n == 0), stop=(n == 8))
                            n += 1
                    sl = slice(r0 * W, r1 * W)
                    flat = ps3.rearrange("p r w -> p (r w)")
                    nc.scalar.activation(conv_sb[:, sl], flat,
                                         AF.Identity, bias=p_c3b,
                                         accum_out=stats_m[:, i:i + 1])
                    sq = plf.tile([C, nr * W], BF16, tag="sq")
                    nc.scalar.activation(sq[:], conv_sb[:, sl],
                                         AF.Square,
                                         accum_out=stats_v[:, i:i + 1])

                stats_a = smid.tile([C, 2], F32)

                def stats_early():
                    nc.vector.tensor_reduce(stats_a[:, 0:1],
                                            stats_m[:, 0:7],
                                            axis=mybir.AxisListType.X,
                                            op=OP.add)
                    nc.vector.tensor_reduce(stats_a[:, 1:2],
                                            stats_v[:, 0:7],
                                            axis=mybir.AxisListType.X,
                                            op=OP.add)
                    nc.sync.dma_start(st_in_a[:], stats_a[:])
                    nc.gpsimd.collective_compute(
                        "AllReduce", OP.add, replica_groups=G8,
                        ins=[st_in_a[:].opt()], outs=[st_out_a[:].opt()])

                y_ps = {}
                bc_q = {}
                PREF = 5

                def bc_issue(q, s):
                    t0, t1 = SEGS[q]
                    SEG = t1 - t0
                    qsl = slice(t0, t1)
                    bbc = plb.tile([DI, SEG], BF16, tag="bbc",
                                   name=f"bbc{q}_{s}")
                    nc.sync.dma_start(
                        bbc[:],
                        bc_dram[s:s + 1, qsl].to_broadcast((DI, SEG)))
                    cbc = plc.tile([DI, SEG], BF16, tag="cbc",
                                   name=f"cbc{q}_{s}")
                    nc.sync.dma_start(
                        cbc[:],
                        bc_dram[DS + s:DS + s + 1, qsl].to_broadcast(
                            (DI, SEG)))
                    bc_q[(q, s)] = (bbc, cbc)

                def seg_prefetch(q):
                    for s in range(PREF):
                        bc_issue(q, s)

                def seg_scan(q, work):
                    t0, t1 = SEGS[q]
                    SEG = t1 - t0
                    qsl = slice(t0, t1)
                    for cix in SEG_CHUNKS[q]:
                        yp = psy.tile([128, CH], F32, tag="yps",
                                      name=f"y{cix}")
                        nc.tensor.matmul(yp[:DI], ident_g[:],
                                         xcd[:, cix * CH:(cix + 1) * CH],
                                         start=True, stop=False)
                        y_ps[cix] = yp
                    for s in range(DS):
                        da = pla.tile([DI, SEG], BF16, tag="da")
                        nc.scalar.activation(da[:], dtv[:, qsl], AF.Exp,
                                             scale=p_A[:, s:s + 1])
                        bbc, cbc = bc_q.pop((q, s))
                        dbx = plx.tile([DI, SEG], BF16, tag="dbx")
                        nc.vector.tensor_mul(dbx[:], dtxc[:, qsl], bbc[:])
                        h = plh.tile([DI, SEG], BF16, tag="h")
                        init = 0.0 if q == 0 else carry[:, s:s + 1]
                        nc.vector.tensor_tensor_scan(h[:], da[:], dbx[:],
                                                     init, op0=OP.mult,
                                                     op1=OP.add)
                        if q < NSEG - 1:
                            # on DVE so the in-order ACT queue of exps is
                            # never blocked behind a scan result
                            nc.vector.tensor_copy(carry[:, s:s + 1],
                                                  h[:, SEG - 1:SEG])
                        g = plg.tile([DI, SEG], BF16, tag="g")
                        nc.vector.tensor_mul(g[:], h[:], cbc[:])
                        for j, cix in enumerate(SEG_CHUNKS[q]):
                            nc.tensor.matmul(
                                y_ps[cix][:DI], ident_g[:],
                                g[:, j * CH:(j + 1) * CH],
                                start=False, stop=(s == DS - 1))
                        if s + PREF < DS:
                            bc_issue(q, s + PREF)
                        if s in work:
                            work[s]()

                # =========== emission ===========
                # all 8 input/gate matmuls first (8 PSUM banks), so the PE
                # never waits on a silu before starting the next chunk
                pend = {}
                pend[0] = front_in(0)
                # dummy 8-core sync: absorbs core launch skew while the
                # front runs, so tail collectives don't pay it
                nc.gpsimd.memset(stats_a[:], 0.0)
                nc.sync.dma_start(sync_in[:], stats_a[:])
                nc.gpsimd.collective_compute(
                    "AllReduce", OP.add, replica_groups=G8,
                    ins=[sync_in[:].opt()], outs=[sync_out[:].opt()])
                pend[1] = front_in(1)
                for c in range(4):
                    if c + 2 < 4:
                        pend[c + 2] = front_in(c + 2)
                    front_mid(c, *pend.pop(c))
                    front_out(c)
                finish_front((0, 1, 2, 3))

                seg_prefetch(0)
                seg_scan(0, {2: lambda: front_chunk(4, None),
                             5: lambda: front_chunk(5, None),
                             8: lambda: front_chunk(6, None),
                             11: lambda: front_chunk(7, (4, 5, 6, 7)),
                             13: lambda: seg_prefetch(1)})
                wave(0)
                seg_scan(1, {6: lambda: conv3_piece(0),
                             9: lambda: conv3_piece(1),
                             12: lambda: conv3_piece(2),
                             13: lambda: seg_prefetch(2)})
                wave(1)
                seg_scan(2, {})
                wave(2)
                # conv pieces 3-6 hide the last exchange's latency on PE
                conv3_piece(3)
                conv3_piece(4)
                conv3_piece(5)
                conv3_piece(6)
                stats_early()
                conv3_piece(7)
                conv3_piece(8)

                # ---- batch stats AllReduces + BN + residual + leaky ----
                tl = smid
                stot = tl.tile([C, 2], F32)
                stot_b = tl.tile([C, 2], F32)
                stats = tl.tile([C, 2], F32)
                nc.vector.tensor_reduce(stats[:, 0:1], stats_m[:, 7:9],
                                        axis=mybir.AxisListType.X, op=OP.add)
                nc.vector.tensor_reduce(stats[:, 1:2], stats_v[:, 7:9],
                                        axis=mybir.AxisListType.X, op=OP.add)
                nc.sync.dma_start(st_in_b[:], stats[:])
                nc.gpsimd.collective_compute(
                    "AllReduce", OP.add, replica_groups=G8,
                    ins=[st_in_b[:].opt()], outs=[st_out_b[:].opt()])
                nc.sync.dma_start(stot[:], st_out_a[:])
                nc.sync.dma_start(stot_b[:], st_out_b[:])
                nc.vector.tensor_add(stot[:], stot[:], stot_b[:])

                # every sample's full conv is present on both pair cores,
                # so the 8-core sum double counts: divide by 2*B*L
                inv = 1.0 / (2.0 * B * L)
                mean = tl.tile([C, 1], F32)
                ex2 = tl.tile([C, 1], F32)
                var = tl.tile([C, 1], F32)
                tmp = tl.tile([C, 1], F32)
                nc.vector.tensor_scalar_mul(mean[:], stot[:, 0:1], inv)
                nc.vector.tensor_scalar_mul(ex2[:], stot[:, 1:2], inv)
                nc.vector.tensor_mul(tmp[:], mean[:], mean[:])
                nc.vector.tensor_sub(var[:], ex2[:], tmp[:])
                # invstd = exp(-0.5*ln(var+eps)) -- ln/exp stay in the
                # loaded table set (no sqrt-set reload on the tail)
                nc.vector.tensor_scalar_add(var[:], var[:], 1e-5)
                nc.scalar.activation(tmp[:], var[:], AF.Ln)
                nc.scalar.activation(tmp[:], tmp[:], AF.Exp, scale=-0.5)
                scal = tl.tile([C, 1], F32)
                shft = tl.tile([C, 1], F32)
                nc.vector.tensor_mul(scal[:], p_bng, tmp[:])
                nc.vector.tensor_mul(tmp[:], mean[:], scal[:])
                nc.vector.tensor_sub(shft[:], p_bnb, tmp[:])

                # bn + residual + leaky relu: out = prelu(conv*scal + res
                # + shft); conv*scal on ACT (per-partition scale), add on
                # DVE at 2x, prelu+shift on ACT straight to f32 out
                for lo in range(0, L, 512):
                    hi = lo + 512
                    bs = plf.tile([C, 512], BF16, tag="bn")
                    nc.scalar.activation(bs[:], conv_sb[:, lo:hi],
                                         AF.Copy, scale=scal[:, 0:1])
                    nc.vector.tensor_add(bs[:], bs[:], res_sb[:, lo:hi])
                    ot = plf.tile([C, 512], F32, tag="ot")
                    nc.scalar.activation(ot[:], bs[:],
                                         AF.Prelu, alpha=0.01,
                                         bias=shft[:, 0:1])
                    nc.sync.dma_start(out_d[:, lo:hi], ot[:])

    nc.compile()
    return nc


_NC = None


def _get_nc():
    global _NC
    if _NC is None:
        _NC = _build()
    return _NC


def _prep_in_maps(inp):
    inp = {k: np.asarray(v, dtype=np.float32) for k, v in inp.items()}
    x = inp["x"]  # (4, 64, 64, 64)
    # full 3x3 conv weights over both direction blocks, [in=128, 9*64]
    c3 = np.zeros((128, 9 * C), np.float32)
    for ky in range(3):
        for kx in range(3):
            c3[:, (ky * 3 + kx) * C:(ky * 3 + kx + 1) * C] = \
                inp["conv_w"][:, :, ky, kx].T
    maps = []
    for core in range(NCORE):
        b, d = core // 2, core % 2
        pre = "m1_" if d == 0 else "m2_"
        in_w = inp[pre + "in_w"]          # (256, 64)
        xproj_w = inp[pre + "xproj_w"]    # (36, 128)
        dt_w = inp[pre + "dt_w"]          # (128, 4)
        conv1_w = inp[pre + "conv_w"]     # (128, 4)

        x_loc = x[b].reshape(C, L)
        if d == 1:
            x_loc = x_loc[:, ::-1]

        # the tanh-form silu leaves xc and z scaled by 2; fold the
        # halves into the consuming weights
        bigproj = 0.5 * (dt_w @ xproj_w[:DTR])    # (128, 128)

        blob_f = np.zeros((128, BF_COLS), np.float32)
        blob_f[:, 0] = inp[pre + "conv_b"]
        blob_f[:, 1] = inp[pre + "dt_b"]
        blob_f[:, 2:18] = -np.exp(inp[pre + "A_log"])
        blob_f[:, 18] = inp[pre + "D"]
        blob_f[:C, 19] = inp["conv_b"]
        blob_f[:C, 20] = inp["res_b"]
        blob_f[:C, 21] = inp["bn_gamma"]
        blob_f[:C, 22] = inp["bn_beta"]
        blob_f[:, 23] = 0.5 * inp[pre + "conv_b"]

        blob_h = np.zeros((128, BH_COLS), np.float32)
        blob_h[:, OFF_C3W:OFF_OWT] = c3
        blob_h[:, OFF_OWT:OFF_BIG] = 0.25 * inp[pre + "out_w"].T
        blob_h[:, OFF_BIG:OFF_BCW] = bigproj.T
        blob_h[:, OFF_BCW:OFF_WK] = 0.5 * xproj_w[DTR:].T
        # fused in-projection + depthwise causal conv:
        # W_k[ch_x, di] = in_w[di, ch_x] * conv1_w[di, k]
        xi_w = in_w[:DI]                  # (128, 64)
        for k in range(DCONV):
            blob_h[:C, OFF_WK + 128 * k:OFF_WK + 128 * (k + 1)] = \
                (xi_w * conv1_w[:, k:k + 1]).T
        blob_h[:C, OFF_ZWT:OFF_ZWT + 128] = in_w[DI:].T
        blob_h[:C, OFF_RWT:OFF_RWT + C] = inp["res_w"].T
        m = {
            "x_loc": np.ascontiguousarray(x_loc).astype(ml_dtypes.bfloat16),
            "blob_f": blob_f,
            "blob_h": blob_h.astype(ml_dtypes.bfloat16),
        }
        maps.append(m)
    return maps


def _run(inputs, trace=False):
    nc = _get_nc()
    maps = _prep_in_maps(inputs)
    res = bass_utils.run_bass_kernel_spmd(
        nc, maps, core_ids=list(range(NCORE)), trace=trace)
    out = np.stack([res.results[2 * b]["out"].reshape(C, H, W)
                    for b in range(B)])
    return out, res


def kernel(**inputs) -> np.ndarray:
    out, _ = _run(inputs, trace=False)
    return out


# revision 53
# speedup vs baseline: 1.1705x; 1.0021x over previous
"""BiMamba block kernel for 8 Trainium2 NeuronCores.

Sharding: core = 2*sample + direction (4 samples x 2 scan directions).
Each core runs the full mamba for its (sample, direction).

The selective scan dominates on DVE (16 states x 4096 cols at ~2.1
ns/col, SBUF-bandwidth limited); the kernel keeps the DVE queue dense
and hides everything else under it:

- Front: fused in-proj + causal depthwise conv (4 accumulating
  matmuls, host-folded weights); silu in tanh form (one table set with
  exp), all 8 input/gate matmuls emitted before any xc-dependent
  matmul so the PE never idles on a silu; chunks 4-7 run between the
  first segment's scan states.
- Scan: 3 segments (2048/1536/512 cols).  Per state: ACT exp -> DVE
  dbx mul -> DVE tensor_tensor_scan -> DVE C mul -> PE PSUM
  accumulate.  B/C rows arrive via DRAM partition-broadcast DMAs
  issued 5 states ahead.
- Exchange: after each segment the out-projection rows are AllGathered
  with the pair core (rank order = [dir0; dir1], matching the
  reference's un-unflipped y2 concat); the full 3x3 conv then runs
  locally -- no collective after the conv.  A dummy 8-core AllReduce
  early in the kernel absorbs core launch skew (~45-60 us).
- Conv is cut into row-pieces such that only rows 55-63 depend on the
  last exchange; BN stats for pieces 0-6 AllReduce early, and invstd
  uses ln/exp so no ACT table load lands on the tail.
"""
import os
import sys

for _p in ("/opt/trn_rl_repo", "/root/.axon_site/_ro/trn_rl_repo"):
    if os.path.isdir(_p):
        if _p not in sys.path:
            sys.path.insert(0, _p)
        break

import ml_dtypes
import numpy as np

# The agent image's antenv lacks axon_hooks; inject it so trace=True can
# capture NTFF profiles (used by test.py for HW timing, not for grading).
try:
    import antenv.axon_hooks  # noqa: F401
except ImportError:
    try:
        import types as _types

        from trn_agent_boot.trn_boot import _ntff_profile_via_ctypes

        _hook = _ntff_profile_via_ctypes("/opt/axon/libaxon_pjrt.so")
        _m = _types.ModuleType("antenv.axon_hooks")
        _m.get_axon_ntff_profile_hook = lambda: _hook
        _m.set_axon_ntff_profile_hook = lambda h: None
        sys.modules["antenv.axon_hooks"] = _m
    except Exception:
        pass

import concourse.bass as bass
import concourse.mybir as mybir
from concourse import bacc
from concourse import bass_utils
from concourse.masks import make_identity
from concourse.tile import TileContext

F32 = mybir.dt.float32
BF16 = mybir.dt.bfloat16
AF = mybir.ActivationFunctionType
OP = mybir.AluOpType

B, C, H, W = 4, 64, 64, 64
L = H * W          # 4096
DI = 128           # d_inner
DS = 16            # d_state
DTR = 4            # dt_rank
DCONV = 4
NCORE = 8
CH = 512           # matmul free-dim chunk
NCH = L // CH      # 8
RPC = CH // W      # output rows per chunk (8)

SEGS = ((0, 2048), (2048, 3584), (3584, 4096))
SEG_CHUNKS = ((0, 1, 2, 3), (4, 5, 6), (7,))
NSEG = len(SEGS)
WAVES = ((0, (0, 1, 2, 3)), (1, (4, 5, 6)), (2, (7,)))
# conv pieces (row ranges): pieces 0-6 need only waves 0-1; pieces 7-8
# (rows 55-63, PSUM-bank-sized) are the only ones gated on wave 2
CONV_PIECES = ((0, 8), (8, 16), (16, 24), (24, 32), (32, 40), (40, 48),
               (48, 55), (55, 63), (63, 64))
NPIECE = len(CONV_PIECES)

# blob_h layout (bf16): c3w | owT | bigT | bcwT | wk0..3 | zwT | rwT
OFF_C3W = 0
OFF_OWT = OFF_C3W + 9 * C
OFF_BIG = OFF_OWT + C
OFF_BCW = OFF_BIG + 128
OFF_WK = OFF_BCW + 32
OFF_ZWT = OFF_WK + 4 * 128
OFF_RWT = OFF_ZWT + 128
BH_COLS = OFF_RWT + C
BF_COLS = 32


def _build():
    nc = bacc.Bacc(target_bir_lowering=False, debug=False, num_devices=NCORE)

    def din(name, shape, dtype=F32):
        return nc.dram_tensor(name, shape, dtype, kind="ExternalInput")

    x_loc = din("x_loc", [C, L], BF16)
    blob_f = din("blob_f", [128, BF_COLS], F32)
    blob_h = din("blob_h", [128, BH_COLS], BF16)

    out_d = nc.dram_tensor("out", [C, L], F32, kind="ExternalOutput")

    with TileContext(nc) as tc:
        with tc.tile_pool(name="pers", bufs=1) as pers:
            # ---- params arrive as two packed blobs ----
            p_bf = pers.tile([128, BF_COLS], F32)
            p_bh = pers.tile([128, BH_COLS], BF16)
            nc.sync.dma_start(p_bf[:], blob_f[:])
            nc.sync.dma_start(p_bh[:], blob_h[:])
            p_c1b = p_bf[:, 0:1]
            p_dtb = p_bf[:, 1:2]
            p_A = p_bf[:, 2:18]
            p_D = p_bf[:, 18:19]
            p_c3b = p_bf[:C, 19:20]
            p_rb = p_bf[:C, 20:21]
            p_bng = p_bf[:C, 21:22]
            p_bnb = p_bf[:C, 22:23]
            p_c1bh = p_bf[:, 23:24]   # 0.5 * conv1 bias
            p_c3w = p_bh[:, OFF_C3W:OFF_OWT]
            p_owT = p_bh[:, OFF_OWT:OFF_BIG]
            p_bigT = p_bh[:, OFF_BIG:OFF_BCW]
            p_bcwT = p_bh[:, OFF_BCW:OFF_WK]
            p_wk = [p_bh[:, OFF_WK + 128 * k:OFF_WK + 128 * (k + 1)]
                    for k in range(DCONV)]
            p_zwT = p_bh[:, OFF_ZWT:OFF_ZWT + 128]
            p_rwT = p_bh[:, OFF_RWT:OFF_RWT + C]

            ident = pers.tile([128, 128], F32)
            make_identity(nc, ident[:])
            ident_g = pers.tile([128, 128], BF16)
            nc.vector.tensor_copy(ident_g[:], ident[:])

            # DRAM staging for B/C rows (DMA partition-broadcast needs a
            # DRAM source)
            bc_dram = nc.dram_tensor("bc_stage", [2 * DS, L], BF16)

            x_pad = pers.tile([64, 3 + L], BF16)
            nc.gpsimd.memset(x_pad[:, 0:3], 0.0)
            # split load so the front chunks start without waiting for
            # the full x (chunk 3's last tap reads through x col 2048)
            nc.sync.dma_start(x_pad[:, 3:3 + 2064], x_loc[:, 0:2064])
            nc.sync.dma_start(x_pad[:, 3 + 2064:3 + L], x_loc[:, 2064:L])

            with tc.tile_pool(name="smid", bufs=1) as smid, \
                 tc.tile_pool(name="ps", bufs=4, space="PSUM") as psp, \
                 tc.tile_pool(name="psy", bufs=4, space="PSUM") as psy, \
                 tc.tile_pool(name="sl_e", bufs=2) as plex, \
                 tc.tile_pool(name="sl_a", bufs=3) as pla, \
                 tc.tile_pool(name="sl_b", bufs=5) as plb, \
                 tc.tile_pool(name="sl_x", bufs=3) as plx, \
                 tc.tile_pool(name="sl_h", bufs=3) as plh, \
                 tc.tile_pool(name="sl_c", bufs=5) as plc, \
                 tc.tile_pool(name="sl_g", bufs=3) as plg, \
                 tc.tile_pool(name="sl_f", bufs=2) as plf, \
                 tc.tile_pool(name="dram", bufs=1, space="DRAM") as dr:
                z_sil = smid.tile([DI, L], BF16)
                dtv = smid.tile([DI, L], BF16)
                dtxc = smid.tile([DI, L], BF16)
                xcd = smid.tile([DI, L], BF16)
                xc = smid.tile([DI, L], BF16)
                carry = smid.tile([DI, DS], F32)

                ympad = smid.tile([128, H + 2, W + 2], BF16)
                nc.gpsimd.memset(ympad[:], 0.0)
                # res rows 0-63, conv rows 64-127 share one tile
                rescv = smid.tile([128, L], BF16)
                res_sb = rescv[0:C]
                conv_sb = rescv[C:128]
                stats_m = smid.tile([C, NPIECE], F32)
                stats_v = smid.tile([C, NPIECE], F32)
                PAIRS = [[0, 1], [2, 3], [4, 5], [6, 7]]
                G8 = [[0, 1, 2, 3, 4, 5, 6, 7]]

                cc_ins, cc_outs = [], []
                for wi, (_, cvs) in enumerate(WAVES):
                    cc_ins.append(dr.tile([C, len(cvs) * CH], BF16,
                                          name=f"cci{wi}"))
                    cc_outs.append(dr.tile([128, len(cvs) * CH], BF16,
                                           name=f"cco{wi}"))
                st_in_a = dr.tile([C, 2], F32, name="st_in_a")
                st_in_b = dr.tile([C, 2], F32, name="st_in_b")
                sync_in = dr.tile([C, 2], F32, name="sync_in")
                st_out_a = nc.dram_tensor("st_out_a", [C, 2], F32,
                                          addr_space="Shared")
                st_out_b = nc.dram_tensor("st_out_b", [C, 2], F32,
                                          addr_space="Shared")
                sync_out = nc.dram_tensor("sync_out", [C, 2], F32,
                                          addr_space="Shared")

                def front_in(c):
                    """Input-dependent matmuls for chunk c (no xc dep)."""
                    ps = psp.tile([128, CH], F32, tag="ps", name=f"fi{c}")
                    for k in range(DCONV):
                        nc.tensor.matmul(ps[:DI], p_wk[k][:C],
                                         x_pad[:, c * CH + k:c * CH + k + CH],
                                         start=(k == 0), stop=(k == DCONV - 1))
                    ps2 = psp.tile([128, CH], F32, tag="ps", name=f"fz{c}")
                    nc.tensor.matmul(ps2[:DI], p_zwT[:C],
                                     x_pad[:, 3 + c * CH:3 + (c + 1) * CH],
                                     start=True, stop=True)
                    return ps, ps2

                def front_mid(c, ps, ps2):
                    """tanh-form silus for chunk c: xc and z_sil come out
                    scaled by 2; the 0.5/0.25 factors are folded into the
                    downstream weights on the host."""
                    sl = slice(c * CH, (c + 1) * CH)
                    th = plf.tile([DI, CH], BF16, tag="th")
                    nc.scalar.activation(th[:], ps[:DI], AF.Tanh,
                                         scale=0.5, bias=p_c1bh)
                    raw = plf.tile([DI, CH], BF16, tag="raw")
                    nc.scalar.activation(raw[:], ps[:DI], AF.Identity,
                                         bias=p_c1b)
                    nc.vector.scalar_tensor_tensor(
                        xc[:, sl], th[:], 1.0, raw[:],
                        op0=OP.add, op1=OP.mult)
                    th2 = plf.tile([DI, CH], BF16, tag="th")
                    nc.scalar.activation(th2[:], ps2[:DI], AF.Tanh,
                                         scale=0.5)
                    raw2 = plf.tile([DI, CH], BF16, tag="raw")
                    nc.scalar.copy(raw2[:], ps2[:DI])
                    nc.vector.scalar_tensor_tensor(
                        z_sil[:, sl], th2[:], 1.0, raw2[:],
                        op0=OP.add, op1=OP.mult)

                def front_out(c):
                    """xc-dependent projections for chunk c."""
                    sl = slice(c * CH, (c + 1) * CH)
                    ps3 = psp.tile([128, CH], F32, tag="ps", name=f"fd{c}")
                    nc.tensor.matmul(ps3[:DI], p_bigT[:], xc[:, sl],
                                     start=True, stop=True)
                    nc.scalar.activation(dtv[:, sl], ps3[:DI], AF.Exp,
                                         bias=p_dtb)
                    ps4 = psp.tile([128, CH], F32, tag="ps", name=f"fb{c}")
                    nc.tensor.matmul(ps4[:2 * DS], p_bcwT[:], xc[:, sl],
                                     start=True, stop=True)
                    bch = plb.tile([2 * DS, CH], BF16, tag="bch")
                    nc.scalar.copy(bch[:], ps4[:2 * DS])
                    nc.sync.dma_start(bc_dram[:, sl], bch[:])

                def finish_front(cs):
                    """softplus (batched Ln) + dtxc + xcd for chunks cs."""
                    hsl = slice(cs[0] * CH, (cs[-1] + 1) * CH)
                    nc.scalar.activation(dtv[:, hsl], dtv[:, hsl], AF.Ln,
                                         bias=1.0)
                    for c in cs:
                        sl = slice(c * CH, (c + 1) * CH)
                        nc.vector.tensor_mul(dtxc[:, sl], dtv[:, sl],
                                             xc[:, sl])
                        nc.scalar.activation(xcd[:, sl], xc[:, sl],
                                             AF.Copy, scale=p_D)

                def front_chunk(c, ln):
                    ps, ps2 = front_in(c)
                    front_mid(c, ps, ps2)
                    front_out(c)
                    if ln:
                        finish_front(ln)

                def wave(wi):
                    """Out-projection + pair AllGather + ympad write +
                    residual for the wave's chunks.  AllGather output is
                    rank-ordered, so both cores get [dir0; dir1]."""
                    cvs = WAVES[wi][1]
                    stage = plex.tile([C, len(cvs) * CH], BF16,
                                      tag="stage", name=f"stage{wi}")
                    for j, cix in enumerate(cvs):
                        sl = slice(cix * CH, (cix + 1) * CH)
                        ssl = slice(j * CH, (j + 1) * CH)
                        yg = plf.tile([DI, CH], BF16, tag="yg")
                        nc.vector.tensor_mul(yg[:], y_ps[cix][:DI],
                                             z_sil[:, sl])
                        po = psp.tile([128, CH], F32, tag="ps",
                                      name=f"po{cix}")
                        nc.tensor.matmul(po[:C], p_owT[:], yg[:],
                                         start=True, stop=True)
                        nc.scalar.copy(stage[:, ssl], po[:C])
                        psr = psp.tile([128, CH], F32, tag="ps",
                                       name=f"rs{cix}")
                        nc.tensor.matmul(psr[:C], p_rwT[:C],
                                         x_pad[:, 3 + cix * CH:
                                               3 + (cix + 1) * CH],
                                         start=True, stop=True)
                        nc.scalar.activation(res_sb[:, sl], psr[:C],
                                             AF.Identity, bias=p_rb)
                    nc.sync.dma_start(cc_ins[wi][:], stage[:])
                    nc.gpsimd.collective_compute(
                        "AllGather", OP.bypass, replica_groups=PAIRS,
                        ins=[cc_ins[wi][:].opt()], outs=[cc_outs[wi][:].opt()])
                    r0 = cvs[0] * RPC
                    nrows = len(cvs) * RPC
                    nc.sync.dma_start(
                        ympad[:, 1 + r0:1 + r0 + nrows, 1:1 + W],
                        cc_outs[wi][:].rearrange("p (r w) -> p r w", w=W))

                def conv3_piece(i):
                    r0, r1 = CONV_PIECES[i]
                    nr = r1 - r0
                    ps = psp.tile([128, nr * W], F32, tag="ps",
                                  name=f"cv{i}")
                    ps3 = ps[:C].rearrange("p (r w) -> p r w", w=W)
                    n = 0
                    for ky in range(3):
                        for kx in range(3):
                            nc.tensor.matmul(
                                ps3[:],
                                p_c3w[:, (ky * 3 + kx) * C:
                                      (ky * 3 + kx + 1) * C],
                                ympad[:, r0 + ky:r0 + ky + nr, kx:kx + W],
                                start=(n == 0), stop=(n == 8))
                            n += 1
                    sl = slice(r0 * W, r1 * W)
                    flat = ps3.rearrange("p r w -> p (r w)")
                    nc.scalar.activation(conv_sb[:, sl], flat,
                                         AF.Identity, bias=p_c3b,
                                         accum_out=stats_m[:, i:i + 1])
                    sq = plf.tile([C, nr * W], BF16, tag="sq")
                    nc.scalar.activation(sq[:], conv_sb[:, sl],
                                         AF.Square,
                                         accum_out=stats_v[:, i:i + 1])

                stats_a = smid.tile([C, 2], F32)

                def stats_early():
                    nc.vector.tensor_reduce(stats_a[:, 0:1],
                                            stats_m[:, 0:7],
                                            axis=mybir.AxisListType.X,
                                            op=OP.add)
                    nc.vector.tensor_reduce(stats_a[:, 1:2],
                                            stats_v[:, 0:7],
                                            axis=mybir.AxisListType.X,
                                            op=OP.add)
                    nc.sync.dma_start(st_in_a[:], stats_a[:])
                    nc.gpsimd.collective_compute(
                        "AllReduce", OP.add, replica_groups=G8,
                        ins=[st_in_a[:].opt()], outs=[st_out_a[:].opt()])

                y_ps = {}
                bc_q = {}
                PREF = 5

                def bc_issue(q, s):
                    t0, t1 = SEGS[q]
                    SEG = t1 - t0
                    qsl = slice(t0, t1)
                    bbc = plb.tile([DI, SEG], BF16, tag="bbc",
                                   name=f"bbc{q}_{s}")
                    nc.sync.dma_start(
                        bbc[:],
                        bc_dram[s:s + 1, qsl].to_broadcast((DI, SEG)))
                    cbc = plc.tile([DI, SEG], BF16, tag="cbc",
                                   name=f"cbc{q}_{s}")
                    nc.sync.dma_start(
                        cbc[:],
                        bc_dram[DS + s:DS + s + 1, qsl].to_broadcast(
                            (DI, SEG)))
                    bc_q[(q, s)] = (bbc, cbc)

                def seg_prefetch(q):
                    for s in range(PREF):
                        bc_issue(q, s)

                def seg_scan(q, work):
                    t0, t1 = SEGS[q]
                    SEG = t1 - t0
                    qsl = slice(t0, t1)
                    for cix in SEG_CHUNKS[q]:
                        yp = psy.tile([128, CH], F32, tag="yps",
                                      name=f"y{cix}")
                        nc.tensor.matmul(yp[:DI], ident_g[:],
                                         xcd[:, cix * CH:(cix + 1) * CH],
                                         start=True, stop=False)
                        y_ps[cix] = yp
                    # software-pipelined state loop: no two adjacent DVE
                    # ops are data-dependent (scan_s | dbx_{s+1} | g_s),
                    # hiding the SBUF write-ack latency between them
                    da_n = pla.tile([DI, SEG], BF16, tag="da")
                    nc.scalar.activation(da_n[:], dtv[:, qsl], AF.Exp,
                                         scale=p_A[:, 0:1])
                    dbx_n = plx.tile([DI, SEG], BF16, tag="dbx")
                    nc.vector.tensor_mul(dbx_n[:], dtxc[:, qsl],
                                         bc_q[(q, 0)][0][:])
                    for s in range(DS):
                        da, dbx = da_n, dbx_n
                        _, cbc = bc_q.pop((q, s))
                        if s + 1 < DS:
                            da_n = pla.tile([DI, SEG], BF16, tag="da")
                            nc.scalar.activation(da_n[:], dtv[:, qsl],
                                                 AF.Exp,
                                                 scale=p_A[:, s + 1:s + 2])
                        h = plh.tile([DI, SEG], BF16, tag="h")
                        init = 0.0 if q == 0 else carry[:, s:s + 1]
                        nc.vector.tensor_tensor_scan(h[:], da[:], dbx[:],
                                                     init, op0=OP.mult,
                                                     op1=OP.add)
                        if s + 1 < DS:
                            dbx_n = plx.tile([DI, SEG], BF16, tag="dbx")
                            nc.vector.tensor_mul(dbx_n[:], dtxc[:, qsl],
                                                 bc_q[(q, s + 1)][0][:])
                        g = plg.tile([DI, SEG], BF16, tag="g")
                        nc.vector.tensor_mul(g[:], h[:], cbc[:])
                        if q < NSEG - 1:
                            nc.vector.tensor_copy(carry[:, s:s + 1],
                                                  h[:, SEG - 1:SEG])
                        for j, cix in enumerate(SEG_CHUNKS[q]):
                            nc.tensor.matmul(
                                y_ps[cix][:DI], ident_g[:],
                                g[:, j * CH:(j + 1) * CH],
                                start=False, stop=(s == DS - 1))
                        if s + PREF < DS:
                            bc_issue(q, s + PREF)
                        if s in work:
                            work[s]()

                # =========== emission ===========
                # all 8 input/gate matmuls first (8 PSUM banks), so the PE
                # never waits on a silu before starting the next chunk
                pend = {}
                pend[0] = front_in(0)
                # dummy 8-core sync: absorbs core launch skew while the
                # front runs, so tail collectives don't pay it
                nc.gpsimd.memset(stats_a[:], 0.0)
                nc.sync.dma_start(sync_in[:], stats_a[:])
                nc.gpsimd.collective_compute(
                    "AllReduce", OP.add, replica_groups=G8,
                    ins=[sync_in[:].opt()], outs=[sync_out[:].opt()])
                pend[1] = front_in(1)
                for c in range(4):
                    if c + 2 < 4:
                        pend[c + 2] = front_in(c + 2)
                    front_mid(c, *pend.pop(c))
                    front_out(c)
                finish_front((0, 1, 2, 3))

                seg_prefetch(0)
                seg_scan(0, {2: lambda: front_chunk(4, None),
                             5: lambda: front_chunk(5, None),
                             8: lambda: front_chunk(6, None),
                             11: lambda: front_chunk(7, (4, 5, 6, 7)),
                             13: lambda: seg_prefetch(1)})
                wave(0)
                seg_scan(1, {6: lambda: conv3_piece(0),
                             9: lambda: conv3_piece(1),
                             12: lambda: conv3_piece(2),
                             13: lambda: seg_prefetch(2)})
                wave(1)
                seg_scan(2, {})
                wave(2)
                # conv pieces 3-6 hide the last exchange's latency on PE
                conv3_piece(3)
                conv3_piece(4)
                conv3_piece(5)
                conv3_piece(6)
                conv3_piece(7)
                conv3_piece(8)

                # ---- batch stats AllReduce + BN + residual + leaky ----
                # single AR: the early sync already absorbed launch skew
                tl = smid
                stot = tl.tile([C, 2], F32)
                stats = tl.tile([C, 2], F32)
                nc.vector.tensor_reduce(stats[:, 0:1], stats_m[:],
                                        axis=mybir.AxisListType.X, op=OP.add)
                nc.vector.tensor_reduce(stats[:, 1:2], stats_v[:],
                                        axis=mybir.AxisListType.X, op=OP.add)
                nc.sync.dma_start(st_in_b[:], stats[:])
                nc.gpsimd.collective_compute(
                    "AllReduce", OP.add, replica_groups=G8,
                    ins=[st_in_b[:].opt()], outs=[st_out_b[:].opt()])
                nc.sync.dma_start(stot[:], st_out_b[:])

                # every sample's full conv is present on both pair cores,
                # so the 8-core sum double counts: divide by 2*B*L
                inv = 1.0 / (2.0 * B * L)
                mean = tl.tile([C, 1], F32)
                ex2 = tl.tile([C, 1], F32)
                var = tl.tile([C, 1], F32)
                tmp = tl.tile([C, 1], F32)
                nc.vector.tensor_scalar_mul(mean[:], stot[:, 0:1], inv)
                nc.vector.tensor_scalar_mul(ex2[:], stot[:, 1:2], inv)
                nc.vector.tensor_mul(tmp[:], mean[:], mean[:])
                nc.vector.tensor_sub(var[:], ex2[:], tmp[:])
                # invstd = exp(-0.5*ln(var+eps)) -- ln/exp stay in the
                # loaded table set (no sqrt-set reload on the tail)
                nc.vector.tensor_scalar_add(var[:], var[:], 1e-5)
                nc.scalar.activation(tmp[:], var[:], AF.Ln)
                nc.scalar.activation(tmp[:], tmp[:], AF.Exp, scale=-0.5)
                scal = tl.tile([C, 1], F32)
                shft = tl.tile([C, 1], F32)
                nc.vector.tensor_mul(scal[:], p_bng, tmp[:])
                nc.vector.tensor_mul(tmp[:], mean[:], scal[:])
                nc.vector.tensor_sub(shft[:], p_bnb, tmp[:])

                # bn + residual + leaky relu: out = prelu(conv*scal + res
                # + shft); conv*scal on ACT (per-partition scale), add on
                # DVE at 2x, prelu+shift on ACT straight to f32 out.
                # Phase-interleaved pairs so adjacent ACT ops are
                # independent (hides write-ack latency).
                for lo in range(0, L, 2048):
                    bss = []
                    for k in range(2):
                        a, b = lo + k * 1024, lo + k * 1024 + 1024
                        bs = plex.tile([C, 1024], BF16, tag="bn")
                        nc.scalar.activation(bs[:], conv_sb[:, a:b],
                                             AF.Copy, scale=scal[:, 0:1])
                        bss.append(bs)
                    for k in range(2):
                        a, b = lo + k * 1024, lo + k * 1024 + 1024
                        nc.vector.tensor_add(bss[k][:], bss[k][:],
                                             res_sb[:, a:b])
                    for k in range(2):
                        a, b = lo + k * 1024, lo + k * 1024 + 1024
                        ot = plex.tile([C, 1024], F32, tag="ot")
                        nc.scalar.activation(ot[:], bss[k][:],
                                             AF.Prelu, alpha=0.01,
                                             bias=shft[:, 0:1])
                        nc.sync.dma_start(out_d[:, a:b], ot[:])

    nc.compile()
    return nc


_NC = None


def _get_nc():
    global _NC
    if _NC is None:
        _NC = _build()
    return _NC


def _prep_in_maps(inp):
    inp = {k: np.asarray(v, dtype=np.float32) for k, v in inp.items()}
    x = inp["x"]  # (4, 64, 64, 64)
    # full 3x3 conv weights over both direction blocks, [in=128, 9*64]
    c3 = np.zeros((128, 9 * C), np.float32)
    for ky in range(3):
        for kx in range(3):
            c3[:, (ky * 3 + kx) * C:(ky * 3 + kx + 1) * C] = \
                inp["conv_w"][:, :, ky, kx].T
    maps = []
    for core in range(NCORE):
        b, d = core // 2, core % 2
        pre = "m1_" if d == 0 else "m2_"
        in_w = inp[pre + "in_w"]          # (256, 64)
        xproj_w = inp[pre + "xproj_w"]    # (36, 128)
        dt_w = inp[pre + "dt_w"]          # (128, 4)
        conv1_w = inp[pre + "conv_w"]     # (128, 4)

        x_loc = x[b].reshape(C, L)
        if d == 1:
            x_loc = x_loc[:, ::-1]

        # the tanh-form silu leaves xc and z scaled by 2; fold the
        # halves into the consuming weights
        bigproj = 0.5 * (dt_w @ xproj_w[:DTR])    # (128, 128)

        blob_f = np.zeros((128, BF_COLS), np.float32)
        blob_f[:, 0] = inp[pre + "conv_b"]
        blob_f[:, 1] = inp[pre + "dt_b"]
        blob_f[:, 2:18] = -np.exp(inp[pre + "A_log"])
        blob_f[:, 18] = inp[pre + "D"]
        blob_f[:C, 19] = inp["conv_b"]
        blob_f[:C, 20] = inp["res_b"]
        blob_f[:C, 21] = inp["bn_gamma"]
        blob_f[:C, 22] = inp["bn_beta"]
        blob_f[:, 23] = 0.5 * inp[pre + "conv_b"]

        blob_h = np.zeros((128, BH_COLS), np.float32)
        blob_h[:, OFF_C3W:OFF_OWT] = c3
        blob_h[:, OFF_OWT:OFF_BIG] = 0.25 * inp[pre + "out_w"].T
        blob_h[:, OFF_BIG:OFF_BCW] = bigproj.T
        blob_h[:, OFF_BCW:OFF_WK] = 0.5 * xproj_w[DTR:].T
        # fused in-projection + depthwise causal conv:
        # W_k[ch_x, di] = in_w[di, ch_x] * conv1_w[di, k]
        xi_w = in_w[:DI]                  # (128, 64)
        for k in range(DCONV):
            blob_h[:C, OFF_WK + 128 * k:OFF_WK + 128 * (k + 1)] = \
                (xi_w * conv1_w[:, k:k + 1]).T
        blob_h[:C, OFF_ZWT:OFF_ZWT + 128] = in_w[DI:].T
        blob_h[:C, OFF_RWT:OFF_RWT + C] = inp["res_w"].T
        m = {
            "x_loc": np.ascontiguousarray(x_loc).astype(ml_dtypes.bfloat16),
            "blob_f": blob_f,
            "blob_h": blob_h.astype(ml_dtypes.bfloat16),
        }
        maps.append(m)
    return maps


def _run(inputs, trace=False):
    nc = _get_nc()
    maps = _prep_in_maps(inputs)
    res = bass_utils.run_bass_kernel_spmd(
        nc, maps, core_ids=list(range(NCORE)), trace=trace)
    out = np.stack([res.results[2 * b]["out"].reshape(C, H, W)
                    for b in range(B)])
    return out, res


def kernel(**inputs) -> np.ndarray:
    out, _ = _run(inputs, trace=False)
    return out


# revision 57
# speedup vs baseline: 1.3510x; 1.1542x over previous
"""BiMamba block kernel for 8 Trainium2 NeuronCores.

Sharding: core = 2*sample + direction (4 samples x 2 scan directions).
Each core runs the full mamba for its (sample, direction).

The selective scan dominates on DVE (16 states x 4096 cols at ~2.1
ns/col, SBUF-bandwidth limited); the kernel keeps the DVE queue dense
and hides everything else under it:

- Front: fused in-proj + causal depthwise conv (4 accumulating
  matmuls, host-folded weights); silu in tanh form (one table set with
  exp), all 8 input/gate matmuls emitted before any xc-dependent
  matmul so the PE never idles on a silu; chunks 4-7 run between the
  first segment's scan states.
- Scan: 3 segments (2048/1536/512 cols).  Per state: ACT exp -> DVE
  dbx mul -> DVE tensor_tensor_scan -> DVE C mul -> PE PSUM
  accumulate.  B/C rows arrive via DRAM partition-broadcast DMAs
  issued 5 states ahead.
- Exchange: after each segment the out-projection rows are AllGathered
  with the pair core (rank order = [dir0; dir1], matching the
  reference's un-unflipped y2 concat); the full 3x3 conv then runs
  locally -- no collective after the conv.  A dummy 8-core AllReduce
  early in the kernel absorbs core launch skew (~45-60 us).
- Conv is cut into row-pieces such that only rows 55-63 depend on the
  last exchange; BN stats for pieces 0-6 AllReduce early, and invstd
  uses ln/exp so no ACT table load lands on the tail.
"""
import os
import sys

for _p in ("/opt/trn_rl_repo", "/root/.axon_site/_ro/trn_rl_repo"):
    if os.path.isdir(_p):
        if _p not in sys.path:
            sys.path.insert(0, _p)
        break

import ml_dtypes
import numpy as np

# The agent image's antenv lacks axon_hooks; inject it so trace=True can
# capture NTFF profiles (used by test.py for HW timing, not for grading).
try:
    import antenv.axon_hooks  # noqa: F401
except ImportError:
    try:
        import types as _types

        from trn_agent_boot.trn_boot import _ntff_profile_via_ctypes

        _hook = _ntff_profile_via_ctypes("/opt/axon/libaxon_pjrt.so")
        _m = _types.ModuleType("antenv.axon_hooks")
        _m.get_axon_ntff_profile_hook = lambda: _hook
        _m.set_axon_ntff_profile_hook = lambda h: None
        sys.modules["antenv.axon_hooks"] = _m
    except Exception:
        pass

import concourse.bass as bass
import concourse.mybir as mybir
from concourse import bacc
from concourse import bass_utils
from concourse.masks import make_identity
from concourse.tile import TileContext

F32 = mybir.dt.float32
BF16 = mybir.dt.bfloat16
AF = mybir.ActivationFunctionType
OP = mybir.AluOpType

B, C, H, W = 4, 64, 64, 64
L = H * W          # 4096
DI = 128           # d_inner
DS = 16            # d_state
DTR = 4            # dt_rank
DCONV = 4
NCORE = 8
CH = 512           # matmul free-dim chunk
NCH = L // CH      # 8
RPC = CH // W      # output rows per chunk (8)

SEGS = ((0, 2560), (2560, 4096))
SEG_CHUNKS = ((0, 1, 2, 3, 4), (5, 6, 7))
NSEG = len(SEGS)
WAVES = ((0, (0, 1, 2, 3, 4)), (1, (5, 6, 7)))
# conv pieces (row ranges): wave 0 delivers out-rows 0-39, so pieces
# 0-4 (conv rows 0-38) run during segment 1; pieces 5-8 follow wave 1
CONV_PIECES = ((0, 8), (8, 16), (16, 24), (24, 32), (32, 38), (38, 46),
               (46, 54), (54, 62), (62, 64))
NPIECE = len(CONV_PIECES)

# blob_h layout (bf16): c3w | owT | bigT | bcwT | wk0..3 | zwT | rwT
OFF_C3W = 0
OFF_OWT = OFF_C3W + 9 * C
OFF_BIG = OFF_OWT + C
OFF_BCW = OFF_BIG + 128
OFF_WK = OFF_BCW + 32
OFF_ZWT = OFF_WK + 4 * 128
OFF_RWT = OFF_ZWT + 128
BH_COLS = OFF_RWT + C
BF_COLS = 32


def _build():
    nc = bacc.Bacc(target_bir_lowering=False, debug=False, num_devices=NCORE)

    def din(name, shape, dtype=F32):
        return nc.dram_tensor(name, shape, dtype, kind="ExternalInput")

    x_loc = din("x_loc", [C, L], BF16)
    blob_f = din("blob_f", [128, BF_COLS], F32)
    blob_h = din("blob_h", [128, BH_COLS], BF16)

    out_d = nc.dram_tensor("out", [C, L], F32, kind="ExternalOutput")

    with TileContext(nc) as tc:
        with tc.tile_pool(name="pers", bufs=1) as pers:
            # ---- params arrive as two packed blobs ----
            p_bf = pers.tile([128, BF_COLS], F32)
            p_bh = pers.tile([128, BH_COLS], BF16)
            nc.sync.dma_start(p_bf[:], blob_f[:])
            nc.sync.dma_start(p_bh[:], blob_h[:])
            p_c1b = p_bf[:, 0:1]
            p_dtb = p_bf[:, 1:2]
            p_A = p_bf[:, 2:18]
            p_D = p_bf[:, 18:19]
            p_c3b = p_bf[:C, 19:20]
            p_rb = p_bf[:C, 20:21]
            p_bng = p_bf[:C, 21:22]
            p_bnb = p_bf[:C, 22:23]
            p_c1bh = p_bf[:, 23:24]   # 0.5 * conv1 bias
            p_c3w = p_bh[:, OFF_C3W:OFF_OWT]
            p_owT = p_bh[:, OFF_OWT:OFF_BIG]
            p_bigT = p_bh[:, OFF_BIG:OFF_BCW]
            p_bcwT = p_bh[:, OFF_BCW:OFF_WK]
            p_wk = [p_bh[:, OFF_WK + 128 * k:OFF_WK + 128 * (k + 1)]
                    for k in range(DCONV)]
            p_zwT = p_bh[:, OFF_ZWT:OFF_ZWT + 128]
            p_rwT = p_bh[:, OFF_RWT:OFF_RWT + C]

            ident = pers.tile([128, 128], F32)
            make_identity(nc, ident[:])
            ident_g = pers.tile([128, 128], BF16)
            nc.vector.tensor_copy(ident_g[:], ident[:])

            # DRAM staging for B/C rows (DMA partition-broadcast needs a
            # DRAM source)
            bc_dram = nc.dram_tensor("bc_stage", [2 * DS, L], BF16)

            x_pad = pers.tile([64, 3 + L], BF16)
            nc.gpsimd.memset(x_pad[:, 0:3], 0.0)
            # split load so the front chunks start without waiting for
            # the full x (chunk 4's last tap reads through x col 2560)
            nc.sync.dma_start(x_pad[:, 3:3 + 2576], x_loc[:, 0:2576])
            nc.sync.dma_start(x_pad[:, 3 + 2576:3 + L], x_loc[:, 2576:L])

            with tc.tile_pool(name="smid", bufs=1) as smid, \
                 tc.tile_pool(name="ps", bufs=3, space="PSUM") as psp, \
                 tc.tile_pool(name="psy", bufs=5, space="PSUM") as psy, \
                 tc.tile_pool(name="sl_e", bufs=2) as plex, \
                 tc.tile_pool(name="sl_a", bufs=2) as pla, \
                 tc.tile_pool(name="sl_b", bufs=5) as plb, \
                 tc.tile_pool(name="sl_x", bufs=2) as plx, \
                 tc.tile_pool(name="sl_h", bufs=2) as plh, \
                 tc.tile_pool(name="sl_c", bufs=5) as plc, \
                 tc.tile_pool(name="sl_g", bufs=2) as plg, \
                 tc.tile_pool(name="sl_f", bufs=2) as plf, \
                 tc.tile_pool(name="dram", bufs=1, space="DRAM") as dr:
                z_sil = smid.tile([DI, L], BF16)
                dtv = smid.tile([DI, L], BF16)
                dtxc = smid.tile([DI, L], BF16)
                xcd = smid.tile([DI, L], BF16)
                xc = smid.tile([DI, L], BF16)
                carry = smid.tile([DI, DS], F32)

                ympad = smid.tile([128, H + 2, W + 2], BF16)
                nc.gpsimd.memset(ympad[:], 0.0)
                # res rows 0-63, conv rows 64-127 share one tile
                rescv = smid.tile([128, L], BF16)
                res_sb = rescv[0:C]
                conv_sb = rescv[C:128]
                stats_m = smid.tile([C, NPIECE], F32)
                stats_v = smid.tile([C, NPIECE], F32)
                PAIRS = [[0, 1], [2, 3], [4, 5], [6, 7]]
                G8 = [[0, 1, 2, 3, 4, 5, 6, 7]]

                cc_ins, cc_outs = [], []
                for wi, (_, cvs) in enumerate(WAVES):
                    cc_ins.append(dr.tile([C, len(cvs) * CH], BF16,
                                          name=f"cci{wi}"))
                    cc_outs.append(dr.tile([128, len(cvs) * CH], BF16,
                                           name=f"cco{wi}"))
                st_in_a = dr.tile([C, 2], F32, name="st_in_a")
                st_in_b = dr.tile([C, 2], F32, name="st_in_b")
                sync_in = dr.tile([C, 2], F32, name="sync_in")
                st_out_a = nc.dram_tensor("st_out_a", [C, 2], F32,
                                          addr_space="Shared")
                st_out_b = nc.dram_tensor("st_out_b", [C, 2], F32,
                                          addr_space="Shared")
                sync_out = nc.dram_tensor("sync_out", [C, 2], F32,
                                          addr_space="Shared")

                def front_in(c):
                    """Input-dependent matmuls for chunk c (no xc dep)."""
                    ps = psp.tile([128, CH], F32, tag="ps", name=f"fi{c}")
                    for k in range(DCONV):
                        nc.tensor.matmul(ps[:DI], p_wk[k][:C],
                                         x_pad[:, c * CH + k:c * CH + k + CH],
                                         start=(k == 0), stop=(k == DCONV - 1))
                    ps2 = psp.tile([128, CH], F32, tag="ps", name=f"fz{c}")
                    nc.tensor.matmul(ps2[:DI], p_zwT[:C],
                                     x_pad[:, 3 + c * CH:3 + (c + 1) * CH],
                                     start=True, stop=True)
                    return ps, ps2

                def front_mid(c, ps, ps2):
                    """tanh-form silus for chunk c: xc and z_sil come out
                    scaled by 2; the 0.5/0.25 factors are folded into the
                    downstream weights on the host."""
                    sl = slice(c * CH, (c + 1) * CH)
                    th = plf.tile([DI, CH], BF16, tag="th")
                    nc.scalar.activation(th[:], ps[:DI], AF.Tanh,
                                         scale=0.5, bias=p_c1bh)
                    raw = plf.tile([DI, CH], BF16, tag="raw")
                    nc.scalar.activation(raw[:], ps[:DI], AF.Identity,
                                         bias=p_c1b)
                    nc.vector.scalar_tensor_tensor(
                        xc[:, sl], th[:], 1.0, raw[:],
                        op0=OP.add, op1=OP.mult)
                    th2 = plf.tile([DI, CH], BF16, tag="th")
                    nc.scalar.activation(th2[:], ps2[:DI], AF.Tanh,
                                         scale=0.5)
                    raw2 = plf.tile([DI, CH], BF16, tag="raw")
                    nc.scalar.copy(raw2[:], ps2[:DI])
                    nc.vector.scalar_tensor_tensor(
                        z_sil[:, sl], th2[:], 1.0, raw2[:],
                        op0=OP.add, op1=OP.mult)

                def front_out(c):
                    """xc-dependent projections for chunk c."""
                    sl = slice(c * CH, (c + 1) * CH)
                    ps3 = psp.tile([128, CH], F32, tag="ps", name=f"fd{c}")
                    nc.tensor.matmul(ps3[:DI], p_bigT[:], xc[:, sl],
                                     start=True, stop=True)
                    nc.scalar.activation(dtv[:, sl], ps3[:DI], AF.Exp,
                                         bias=p_dtb)
                    ps4 = psp.tile([128, CH], F32, tag="ps", name=f"fb{c}")
                    nc.tensor.matmul(ps4[:2 * DS], p_bcwT[:], xc[:, sl],
                                     start=True, stop=True)
                    bch = plb.tile([2 * DS, CH], BF16, tag="bch")
                    nc.scalar.copy(bch[:], ps4[:2 * DS])
                    nc.sync.dma_start(bc_dram[:, sl], bch[:])

                def finish_front(cs):
                    """softplus (batched Ln) + dtxc + xcd for chunks cs."""
                    hsl = slice(cs[0] * CH, (cs[-1] + 1) * CH)
                    nc.scalar.activation(dtv[:, hsl], dtv[:, hsl], AF.Ln,
                                         bias=1.0)
                    for c in cs:
                        sl = slice(c * CH, (c + 1) * CH)
                        nc.vector.tensor_mul(dtxc[:, sl], dtv[:, sl],
                                             xc[:, sl])
                        nc.scalar.activation(xcd[:, sl], xc[:, sl],
                                             AF.Copy, scale=p_D)

                def front_chunk(c, ln):
                    ps, ps2 = front_in(c)
                    front_mid(c, ps, ps2)
                    front_out(c)
                    if ln:
                        finish_front(ln)

                def wave(wi):
                    """Out-projection + pair AllGather + ympad write +
                    residual for the wave's chunks.  AllGather output is
                    rank-ordered, so both cores get [dir0; dir1]."""
                    cvs = WAVES[wi][1]
                    stage = plex.tile([C, len(cvs) * CH], BF16,
                                      tag="stage", name=f"stage{wi}")
                    for j, cix in enumerate(cvs):
                        sl = slice(cix * CH, (cix + 1) * CH)
                        ssl = slice(j * CH, (j + 1) * CH)
                        yg = plf.tile([DI, CH], BF16, tag="yg")
                        nc.vector.tensor_mul(yg[:], y_ps[cix][:DI],
                                             z_sil[:, sl])
                        po = psp.tile([128, CH], F32, tag="ps",
                                      name=f"po{cix}")
                        nc.tensor.matmul(po[:C], p_owT[:], yg[:],
                                         start=True, stop=True)
                        nc.scalar.copy(stage[:, ssl], po[:C])
                        psr = psp.tile([128, CH], F32, tag="ps",
                                       name=f"rs{cix}")
                        nc.tensor.matmul(psr[:C], p_rwT[:C],
                                         x_pad[:, 3 + cix * CH:
                                               3 + (cix + 1) * CH],
                                         start=True, stop=True)
                        nc.scalar.activation(res_sb[:, sl], psr[:C],
                                             AF.Identity, bias=p_rb)
                    nc.sync.dma_start(cc_ins[wi][:], stage[:])
                    nc.gpsimd.collective_compute(
                        "AllGather", OP.bypass, replica_groups=PAIRS,
                        ins=[cc_ins[wi][:].opt()], outs=[cc_outs[wi][:].opt()])
                    r0 = cvs[0] * RPC
                    nrows = len(cvs) * RPC
                    nc.sync.dma_start(
                        ympad[:, 1 + r0:1 + r0 + nrows, 1:1 + W],
                        cc_outs[wi][:].rearrange("p (r w) -> p r w", w=W))

                def conv3_piece(i):
                    r0, r1 = CONV_PIECES[i]
                    nr = r1 - r0
                    ps = psp.tile([128, nr * W], F32, tag="ps",
                                  name=f"cv{i}")
                    ps3 = ps[:C].rearrange("p (r w) -> p r w", w=W)
                    n = 0
                    for ky in range(3):
                        for kx in range(3):
                            nc.tensor.matmul(
                                ps3[:],
                                p_c3w[:, (ky * 3 + kx) * C:
                                      (ky * 3 + kx + 1) * C],
                                ympad[:, r0 + ky:r0 + ky + nr, kx:kx + W],
                                start=(n == 0), stop=(n == 8))
                            n += 1
                    sl = slice(r0 * W, r1 * W)
                    flat = ps3.rearrange("p r w -> p (r w)")
                    nc.scalar.activation(conv_sb[:, sl], flat,
                                         AF.Identity, bias=p_c3b,
                                         accum_out=stats_m[:, i:i + 1])
                    sq = plf.tile([C, nr * W], BF16, tag="sq")
                    nc.scalar.activation(sq[:], conv_sb[:, sl],
                                         AF.Square,
                                         accum_out=stats_v[:, i:i + 1])

                stats_a = smid.tile([C, 2], F32)

                def stats_early():
                    nc.vector.tensor_reduce(stats_a[:, 0:1],
                                            stats_m[:, 0:7],
                                            axis=mybir.AxisListType.X,
                                            op=OP.add)
                    nc.vector.tensor_reduce(stats_a[:, 1:2],
                                            stats_v[:, 0:7],
                                            axis=mybir.AxisListType.X,
                                            op=OP.add)
                    nc.sync.dma_start(st_in_a[:], stats_a[:])
                    nc.gpsimd.collective_compute(
                        "AllReduce", OP.add, replica_groups=G8,
                        ins=[st_in_a[:].opt()], outs=[st_out_a[:].opt()])

                y_ps = {}
                bc_q = {}
                PREF = 5

                def bc_issue(q, s):
                    t0, t1 = SEGS[q]
                    SEG = t1 - t0
                    qsl = slice(t0, t1)
                    bbc = plb.tile([DI, SEG], BF16, tag="bbc",
                                   name=f"bbc{q}_{s}")
                    nc.sync.dma_start(
                        bbc[:],
                        bc_dram[s:s + 1, qsl].to_broadcast((DI, SEG)))
                    cbc = plc.tile([DI, SEG], BF16, tag="cbc",
                                   name=f"cbc{q}_{s}")
                    nc.sync.dma_start(
                        cbc[:],
                        bc_dram[DS + s:DS + s + 1, qsl].to_broadcast(
                            (DI, SEG)))
                    bc_q[(q, s)] = (bbc, cbc)

                def seg_prefetch(q):
                    for s in range(PREF):
                        bc_issue(q, s)

                def seg_scan(q, work):
                    t0, t1 = SEGS[q]
                    SEG = t1 - t0
                    qsl = slice(t0, t1)
                    for cix in SEG_CHUNKS[q]:
                        yp = psy.tile([128, CH], F32, tag="yps",
                                      name=f"y{cix}")
                        nc.tensor.matmul(yp[:DI], ident_g[:],
                                         xcd[:, cix * CH:(cix + 1) * CH],
                                         start=True, stop=False)
                        y_ps[cix] = yp
                    # software-pipelined state loop: no two adjacent DVE
                    # ops are data-dependent (scan_s | dbx_{s+1} | g_s),
                    # hiding the SBUF write-ack latency between them
                    da_n = pla.tile([DI, SEG], BF16, tag="da")
                    nc.scalar.activation(da_n[:], dtv[:, qsl], AF.Exp,
                                         scale=p_A[:, 0:1])
                    dbx_n = plx.tile([DI, SEG], BF16, tag="dbx")
                    nc.vector.tensor_mul(dbx_n[:], dtxc[:, qsl],
                                         bc_q[(q, 0)][0][:])
                    for s in range(DS):
                        da, dbx = da_n, dbx_n
                        _, cbc = bc_q.pop((q, s))
                        if s + 1 < DS:
                            da_n = pla.tile([DI, SEG], BF16, tag="da")
                            nc.scalar.activation(da_n[:], dtv[:, qsl],
                                                 AF.Exp,
                                                 scale=p_A[:, s + 1:s + 2])
                        h = plh.tile([DI, SEG], BF16, tag="h")
                        init = 0.0 if q == 0 else carry[:, s:s + 1]
                        nc.vector.tensor_tensor_scan(h[:], da[:], dbx[:],
                                                     init, op0=OP.mult,
                                                     op1=OP.add)
                        if s + 1 < DS:
                            dbx_n = plx.tile([DI, SEG], BF16, tag="dbx")
                            nc.vector.tensor_mul(dbx_n[:], dtxc[:, qsl],
                                                 bc_q[(q, s + 1)][0][:])
                        g = plg.tile([DI, SEG], BF16, tag="g")
                        nc.vector.tensor_mul(g[:], h[:], cbc[:])
                        if q < NSEG - 1:
                            nc.vector.tensor_copy(carry[:, s:s + 1],
                                                  h[:, SEG - 1:SEG])
                        for j, cix in enumerate(SEG_CHUNKS[q]):
                            nc.tensor.matmul(
                                y_ps[cix][:DI], ident_g[:],
                                g[:, j * CH:(j + 1) * CH],
                                start=False, stop=(s == DS - 1))
                        if s + PREF < DS:
                            bc_issue(q, s + PREF)
                        if s in work:
                            work[s]()

                # =========== emission ===========
                # all 8 input/gate matmuls first (8 PSUM banks), so the PE
                # never waits on a silu before starting the next chunk
                pend = {}
                pend[0] = front_in(0)
                # dummy 8-core sync: absorbs core launch skew while the
                # front runs, so tail collectives don't pay it
                nc.gpsimd.memset(stats_a[:], 0.0)
                nc.sync.dma_start(sync_in[:], stats_a[:])
                nc.gpsimd.collective_compute(
                    "AllReduce", OP.add, replica_groups=G8,
                    ins=[sync_in[:].opt()], outs=[sync_out[:].opt()])
                pend[1] = front_in(1)
                for c in range(5):
                    if c + 2 < 5:
                        pend[c + 2] = front_in(c + 2)
                    front_mid(c, *pend.pop(c))
                    front_out(c)
                finish_front((0, 1, 2, 3, 4))

                seg_prefetch(0)
                seg_scan(0, {2: lambda: front_chunk(5, None),
                             6: lambda: front_chunk(6, None),
                             10: lambda: front_chunk(7, (5, 6, 7)),
                             13: lambda: seg_prefetch(1)})
                wave(0)
                seg_scan(1, {7: lambda: conv3_piece(0),
                             9: lambda: conv3_piece(1),
                             11: lambda: conv3_piece(2),
                             13: lambda: conv3_piece(3),
                             15: lambda: conv3_piece(4)})
                wave(1)
                # conv pieces 5-8 hide the last exchange's latency on PE
                conv3_piece(5)
                conv3_piece(6)
                conv3_piece(7)
                conv3_piece(8)

                # ---- batch stats AllReduce + BN + residual + leaky ----
                # single AR: the early sync already absorbed launch skew
                tl = smid
                stot = tl.tile([C, 2], F32)
                stats = tl.tile([C, 2], F32)
                nc.vector.tensor_reduce(stats[:, 0:1], stats_m[:],
                                        axis=mybir.AxisListType.X, op=OP.add)
                nc.vector.tensor_reduce(stats[:, 1:2], stats_v[:],
                                        axis=mybir.AxisListType.X, op=OP.add)
                nc.sync.dma_start(st_in_b[:], stats[:])
                nc.gpsimd.collective_compute(
                    "AllReduce", OP.add, replica_groups=G8,
                    ins=[st_in_b[:].opt()], outs=[st_out_b[:].opt()])
                nc.sync.dma_start(stot[:], st_out_b[:])

                # every sample's full conv is present on both pair cores,
                # so the 8-core sum double counts: divide by 2*B*L
                inv = 1.0 / (2.0 * B * L)
                mean = tl.tile([C, 1], F32)
                ex2 = tl.tile([C, 1], F32)
                var = tl.tile([C, 1], F32)
                tmp = tl.tile([C, 1], F32)
                nc.vector.tensor_scalar_mul(mean[:], stot[:, 0:1], inv)
                nc.vector.tensor_scalar_mul(ex2[:], stot[:, 1:2], inv)
                nc.vector.tensor_mul(tmp[:], mean[:], mean[:])
                nc.vector.tensor_sub(var[:], ex2[:], tmp[:])
                # invstd = exp(-0.5*ln(var+eps)) -- ln/exp stay in the
                # loaded table set (no sqrt-set reload on the tail)
                nc.vector.tensor_scalar_add(var[:], var[:], 1e-5)
                nc.scalar.activation(tmp[:], var[:], AF.Ln)
                nc.scalar.activation(tmp[:], tmp[:], AF.Exp, scale=-0.5)
                scal = tl.tile([C, 1], F32)
                shft = tl.tile([C, 1], F32)
                nc.vector.tensor_mul(scal[:], p_bng, tmp[:])
                nc.vector.tensor_mul(tmp[:], mean[:], scal[:])
                nc.vector.tensor_sub(shft[:], p_bnb, tmp[:])

                # bn + residual + leaky relu: out = prelu(conv*scal + res
                # + shft); conv*scal on ACT (per-partition scale), add on
                # DVE at 2x, prelu+shift on ACT straight to f32 out.
                # Phase-interleaved pairs so adjacent ACT ops are
                # independent (hides write-ack latency).
                for lo in range(0, L, 2048):
                    bss = []
                    for k in range(2):
                        a, b = lo + k * 1024, lo + k * 1024 + 1024
                        bs = plex.tile([C, 1024], BF16, tag="bn")
                        nc.scalar.activation(bs[:], conv_sb[:, a:b],
                                             AF.Copy, scale=scal[:, 0:1])
                        bss.append(bs)
                    for k in range(2):
                        a, b = lo + k * 1024, lo + k * 1024 + 1024
                        nc.vector.tensor_add(bss[k][:], bss[k][:],
                                             res_sb[:, a:b])
                    for k in range(2):
                        a, b = lo + k * 1024, lo + k * 1024 + 1024
                        ot = plex.tile([C, 1024], F32, tag="ot")
                        nc.scalar.activation(ot[:], bss[k][:],
                                             AF.Prelu, alpha=0.01,
                                             bias=shft[:, 0:1])
                        nc.sync.dma_start(out_d[:, a:b], ot[:])

    nc.compile()
    return nc


_NC = None


def _get_nc():
    global _NC
    if _NC is None:
        _NC = _build()
    return _NC


def _prep_in_maps(inp):
    inp = {k: np.asarray(v, dtype=np.float32) for k, v in inp.items()}
    x = inp["x"]  # (4, 64, 64, 64)
    # full 3x3 conv weights over both direction blocks, [in=128, 9*64]
    c3 = np.zeros((128, 9 * C), np.float32)
    for ky in range(3):
        for kx in range(3):
            c3[:, (ky * 3 + kx) * C:(ky * 3 + kx + 1) * C] = \
                inp["conv_w"][:, :, ky, kx].T
    maps = []
    for core in range(NCORE):
        b, d = core // 2, core % 2
        pre = "m1_" if d == 0 else "m2_"
        in_w = inp[pre + "in_w"]          # (256, 64)
        xproj_w = inp[pre + "xproj_w"]    # (36, 128)
        dt_w = inp[pre + "dt_w"]          # (128, 4)
        conv1_w = inp[pre + "conv_w"]     # (128, 4)

        x_loc = x[b].reshape(C, L)
        if d == 1:
            x_loc = x_loc[:, ::-1]

        # the tanh-form silu leaves xc and z scaled by 2; fold the
        # halves into the consuming weights
        bigproj = 0.5 * (dt_w @ xproj_w[:DTR])    # (128, 128)

        blob_f = np.zeros((128, BF_COLS), np.float32)
        blob_f[:, 0] = inp[pre + "conv_b"]
        blob_f[:, 1] = inp[pre + "dt_b"]
        blob_f[:, 2:18] = -np.exp(inp[pre + "A_log"])
        blob_f[:, 18] = inp[pre + "D"]
        blob_f[:C, 19] = inp["conv_b"]
        blob_f[:C, 20] = inp["res_b"]
        blob_f[:C, 21] = inp["bn_gamma"]
        blob_f[:C, 22] = inp["bn_beta"]
        blob_f[:, 23] = 0.5 * inp[pre + "conv_b"]

        blob_h = np.zeros((128, BH_COLS), np.float32)
        blob_h[:, OFF_C3W:OFF_OWT] = c3
        blob_h[:, OFF_OWT:OFF_BIG] = 0.25 * inp[pre + "out_w"].T
        blob_h[:, OFF_BIG:OFF_BCW] = bigproj.T
        blob_h[:, OFF_BCW:OFF_WK] = 0.5 * xproj_w[DTR:].T
        # fused in-projection + depthwise causal conv:
        # W_k[ch_x, di] = in_w[di, ch_x] * conv1_w[di, k]
        xi_w = in_w[:DI]                  # (128, 64)
        for k in range(DCONV):
            blob_h[:C, OFF_WK + 128 * k:OFF_WK + 128 * (k + 1)] = \
                (xi_w * conv1_w[:, k:k + 1]).T
        blob_h[:C, OFF_ZWT:OFF_ZWT + 128] = in_w[DI:].T
        blob_h[:C, OFF_RWT:OFF_RWT + C] = inp["res_w"].T
        m = {
            "x_loc": np.ascontiguousarray(x_loc).astype(ml_dtypes.bfloat16),
            "blob_f": blob_f,
            "blob_h": blob_h.astype(ml_dtypes.bfloat16),
        }
        maps.append(m)
    return maps


def _run(inputs, trace=False):
    nc = _get_nc()
    maps = _prep_in_maps(inputs)
    res = bass_utils.run_bass_kernel_spmd(
        nc, maps, core_ids=list(range(NCORE)), trace=trace)
    out = np.stack([res.results[2 * b]["out"].reshape(C, H, W)
                    for b in range(B)])
    return out, res


def kernel(**inputs) -> np.ndarray:
    out, _ = _run(inputs, trace=False)
    return out
